# revision 63
# baseline (speedup 1.0000x reference)
"""MultiHeadedAttention Trainium2 kernel (8-core SPMD, data-parallel).

Sharding: 8 cores = (batch b in 0..3) x (query half in 0..1). Each core
computes out[b, half*1024:(half+1)*1024, :] independently - no collectives.

v2 pipeline (vs v1: no DRAM staging, per-head-pair proj/attention overlap):
  - host casts x/mask to bf16; device DMA-transposes them straight from the
    input DRAM tensors into SBUF (xbar transpose, 2-byte dtype). All
    transposes on the SP queue only (dual-queue transposes corrupt on HW)
    and phase-separated from DMA copies (Tile serializes copy<->transpose
    pairs ~4us each for the xbar hazard). Every DMA gets its own tile
    (multi-DMA fills of one tile serialize on WAW semaphores).
  - V projection runs first (PE-dense) while k/q/mask transposes stream in
  - per head pair hp: K/Q projection chunks for hp+1 are emitted inside the
    attention j-loop of hp (512-wide psums, emitted after the scores), so PE
    never drains while ACT (exp) is busy
  - attention j-loop is software-pipelined: iteration j emits PV halves of
    j-1 (deps long ready), a proj chunk, then scores/exp/mask for j+1 so
    the in-order PE queue never stalls on exp's psum-slot release
  - scoresT [k,q] psum -> exp on ScalarE -> mask multiply on DVE (bf16 2x
    mode) -> PV accumulate (Z via trailing ones column of v)
  - finalize: Z-row copy on ACT, PE-broadcast Z, reciprocal_approx_fast,
    multiply, DMA hop into xattnT [dm, q]
  - epilogue out = xattnT.T @ WoT + R, R = bv@WoT + bo, bf16 out staging
    cast to f32 by SWDGE during the store
"""
import numpy as np
import ml_dtypes

import concourse.bass as bass
import concourse.mybir as mybir
import concourse.tile as tile
from concourse import bacc
from concourse.bass_utils import run_bass_kernel_spmd

F32 = mybir.dt.float32
BF16 = mybir.dt.bfloat16
FP8 = mybir.dt.float8e4
AF = mybir.ActivationFunctionType
ALU = mybir.AluOpType

N_CORES = 8
DK = 64


def slices(total, chunk):
    return [(s, min(chunk, total - s)) for s in range(0, total, chunk)]


class Cfg:
    def __init__(self, SQ=1024, SK=2048, DM=1024, H=16, max_stage=5):
        assert DM % 128 == 0 and SK % 128 == 0 and SQ % 128 == 0 and H % 2 == 0
        self.SQ, self.SK, self.DM, self.H = SQ, SK, DM, H
        self.KT = DM // 128          # dm contraction chunks
        self.HP = H // 2             # head pairs
        self.NJ = SK // 128          # Sk tiles
        self.SQS = min(1024, SQ)     # attention Sq slice width (2 psum banks)
        self.max_stage = max_stage
        assert SQ % self.SQS == 0
        assert H * DK == DM


def emit_kernel(tc, cfg, io):
    nc = tc.nc
    C = cfg
    xq, xk, xv, msk = io["xq"], io["xk"], io["xv"], io["mask"]
    w_dram = {"q": io["wqt"], "k": io["wkt"], "v": io["wvt"], "o": io["wot"]}
    bql, bkl, bvl, bo_row = io["bql"], io["bkl"], io["bvl"], io["bo_row"]
    out = io["out"]
    PS_F = max(C.SQS, 512)

    pools = {}

    def open_pool(name, bufs=1, space="SBUF"):
        pools[name] = tc.alloc_tile_pool(name=name, bufs=bufs, space=space)
        return pools[name]

    persist = open_pool("persist", 1)
    rot = open_pool("rot", 1)
    ps_s = open_pool("ps_s", 2, space="PSUM")
    ps_pv = open_pool("ps_pv", 2, space="PSUM")
    work = open_pool("work", 1)
    poolA = open_pool("A", 1)   # LIFO: A on top so it can release mid-kernel

    # ---------------- persistent tiles ----------------
    # per-DMA-chunk tiles: a multi-DMA fill of ONE tile serializes on Tile's
    # same-tile WAW semaphores (~5us per DMA round trip), so every DMA gets
    # its own tile
    v_sb = persist.tile([128, C.NJ * C.H * 65], BF16, name="v_sb")
    maskT_t = [persist.tile([128, C.SQ], BF16, name=f"maskT{j}")
               for j in range(C.NJ)]
    xkT_t = [persist.tile([128, C.SK], BF16, name=f"xkT{kt}")
             for kt in range(C.KT)]
    xqT_t = [persist.tile([128, C.SQ], BF16, name=f"xqT{kt}")
             for kt in range(C.KT)]
    R_sb = persist.tile([128, C.DM], BF16, name="R_sb")
    bql_sb = persist.tile([128, C.HP], F32, name="bql_sb")
    bkl_sb = persist.tile([128, C.HP], F32, name="bkl_sb")
    bvl_sb = persist.tile([128, C.KT], BF16, name="bvl_sb")
    bo_sb = persist.tile([1, C.DM], BF16, name="bo_sb")
    onesb = persist.tile([1, 128], BF16, name="onesb")
    onesf = persist.tile([1, 128], F32, name="onesf")

    xvT_t = {(kt, h): poolA.tile([128, C.SK // 2], BF16, name=f"xvT{kt}_{h}")
             for kt in range(C.KT) for h in range(2)}
    wv_t = [poolA.tile([128, C.DM], BF16, name=f"wv{kt}") for kt in range(C.KT)]

    v_view = v_sb.rearrange("p (j h c) -> p j h c", j=C.NJ, c=65)

    # rotating double-buffered tiles, memoized so every use of (kind, hp)
    # shares one handle (a second pool.tile() call would alias a NEW tile
    # into the slot instead of reading what the projection wrote)
    _rot_tiles = {}

    def _rot(kind, hp, shape, nbuf=2):
        key = (kind, hp)
        if key not in _rot_tiles:
            _rot_tiles[key] = rot.tile(shape, BF16, name=f"{kind}{hp % nbuf}",
                                       tag=f"{kind}{hp % nbuf}")
        return _rot_tiles[key]

    def kT_buf(hp):
        return _rot("kT", hp, [128, C.SK])

    def qT_buf(hp):
        return _rot("qT", hp, [128, C.SQ])

    # single-buffered: wkh(hp) is fully consumed by proj(hp) during
    # attention(hp-1), before wkh(hp+1)'s load is issued
    def wkh_buf(hp):
        return _rot("wk", hp, [128, C.KT * 128], nbuf=1)

    def wqh_buf(hp):
        return _rot("wq", hp, [128, C.KT * 128], nbuf=1)

    # ---------------- prologue DMAs ----------------
    nc.gpsimd.dma_start(bql_sb[:], bql[:])
    nc.gpsimd.dma_start(bkl_sb[:], bkl[:])
    nc.gpsimd.dma_start(bvl_sb[:], bvl[:])
    nc.gpsimd.dma_start(bo_sb[:], bo_row[:])
    nc.vector.memset(onesb[:], 1.0)
    nc.vector.memset(onesf[:], 1.0)
    nc.vector.memset(v_view[:, :, :, 64:65], 1.0)

    # DMA phase discipline: copies and transposes must not coexist in the
    # schedule window (Tile serializes every DMACopy<->DmaTranspose pair,
    # ~4us each, to dodge a real HW xbar deadlock). All prologue copies
    # first, then all transposes (SP queue only - dual-queue transposes
    # corrupt data on HW).
    def load_wh(dst, name_w, hp):
        # [1024, 128] column slice -> [128, KT*128] (kt-blocked), one DMA
        nc.sync.dma_start(
            dst.rearrange("p (k c) -> p k c", k=C.KT),
            w_dram[name_w][:, hp * 128:(hp + 1) * 128].rearrange(
                "(k p) c -> p k c", p=128),
        )

    # copy phase split across SWDGE + ACT HWDGE so it drains in ~5us and the
    # transpose phase (SP) starts sooner
    for kt in range(C.KT):
        q = nc.gpsimd if kt % 2 == 0 else nc.scalar
        q.dma_start(wv_t[kt][:], w_dram["v"][kt * 128:(kt + 1) * 128, :])
    load_wh(wkh_buf(0), "k", 0)
    load_wh(wqh_buf(0), "q", 0)

    # xvT in Sk-halves (own tile each): V proj's first k-tiles unblock
    # after 8 transposes instead of all 8 full-Sk ones
    for h, (hs, hw) in enumerate(slices(C.SK, C.SK // 2)):
        for kt in range(C.KT):
            nc.sync.dma_start(
                xvT_t[(kt, h)][:],
                xv[hs:hs + hw, kt * 128:(kt + 1) * 128], transpose=True)
    for kt in range(C.KT):
        nc.sync.dma_start(
            xkT_t[kt][:], xk[:, kt * 128:(kt + 1) * 128], transpose=True)
    for kt in range(C.KT):
        nc.sync.dma_start(
            xqT_t[kt][:], xq[:, kt * 128:(kt + 1) * 128], transpose=True)

    # ---------------- V projection (PE-dense pipeline fill) ----------------
    def v_proj_j(j):
        ps = ps_s.tile([128, C.DM], F32, name="ps_v", tag="s",
                       padded_shape=[128, PS_F])
        h, jo = divmod(j * 128, C.SK // 2)
        for (ds_, dw) in slices(C.DM, 512):
            for kt in range(C.KT):
                nc.tensor.matmul(
                    ps[:, ds_:ds_ + dw],
                    xvT_t[(kt, h)][:, jo:jo + 128],
                    wv_t[kt][:, ds_:ds_ + dw],
                    start=(kt == 0), stop=(kt == C.KT - 1),
                )
        nc.vector.tensor_copy(
            v_view[:, j, :, 0:64],
            ps.rearrange("p (h c) -> p h c", c=DK),
        )

    for j in range(C.NJ):
        v_proj_j(j)

    # mask: bf16 transposes from DRAM, one per-j tile each (no WAW chain)
    for j in range(C.NJ):
        nc.sync.dma_start(maskT_t[j][:], msk[:, j * 128:(j + 1) * 128],
                          transpose=True)

    poolA.release()
    del pools["A"]
    poolB = open_pool("B", 1)
    xattnT = poolB.tile([128, C.HP * C.SQ], BF16, name="xattnT")
    wo_t = [poolB.tile([128, C.DM], BF16, name=f"wo{kt}") for kt in range(C.KT)]
    for kt in range(C.KT):
        nc.gpsimd.dma_start(wo_t[kt][:], w_dram["o"][kt * 128:(kt + 1) * 128, :])

    if C.max_stage <= 2:
        for pl in reversed(list(pools.values())):
            pl.release()
        return

    # ---------------- projections (emitted per head pair) ----------------
    # projection psums are 512 wide: short "s"-slot holds so the scores/exp
    # rotation stalls at most ~1.7us when a proj tile steals a slot
    def _proj_chunk(xT_t, w, dst, bias_col, ns, nw):
        ps = ps_s.tile([128, nw], F32, name="ps_kp", tag="s",
                       padded_shape=[128, PS_F])
        for kt in range(C.KT):
            nc.tensor.matmul(
                ps[:],
                w[:, kt * 128:(kt + 1) * 128],
                xT_t[kt][:, ns: ns + nw],
                start=(kt == 0), stop=(kt == C.KT - 1),
            )
        nc.vector.tensor_scalar_add(out=dst[:, ns:ns + nw], in0=ps[:],
                                    scalar1=bias_col)

    def proj_chunks(hp, kT, wk, qT, wq):
        """Closures emitting one 512-wide projection chunk each (4 K + 2 Q)."""
        out = []
        for (ns, nw) in slices(C.SK, 512):
            out.append(lambda ns=ns, nw=nw: _proj_chunk(
                xkT_t, wk, kT, bkl_sb[:, hp:hp + 1], ns, nw))
        for (ns, nw) in slices(C.SQ, 512):
            out.append(lambda ns=ns, nw=nw: _proj_chunk(
                xqT_t, wq, qT, bql_sb[:, hp:hp + 1], ns, nw))
        return out

    def k_proj(hp, kT, wk):
        for (ns, nw) in slices(C.SK, 512):
            _proj_chunk(xkT_t, wk, kT, bkl_sb[:, hp:hp + 1], ns, nw)

    def q_proj(hp, qT, wq):
        for (ns, nw) in slices(C.SQ, 512):
            _proj_chunk(xqT_t, wq, qT, bql_sb[:, hp:hp + 1], ns, nw)

    def emit_R():
        # R = bv@WoT + bo, PE-broadcast to 128 rows
        psR = ps_s.tile([1, C.DM], F32, name="psR", tag="s",
                        padded_shape=[128, PS_F])
        for (ns, nw) in slices(C.DM, 512):
            for kt in range(C.KT):
                nc.tensor.matmul(
                    psR[0:1, ns:ns + nw], bvl_sb[:, kt:kt + 1],
                    wo_t[kt][:, ns:ns + nw],
                    start=(kt == 0), stop=(kt == C.KT - 1),
                )
        Rrow = work.tile([1, C.DM], BF16, name="Rrow", tag="zrow", bufs=1,
                         padded_shape=[1, max(C.DM, C.SQS)])
        nc.vector.tensor_tensor(out=Rrow[:], in0=psR[:], in1=bo_sb[:],
                                op=ALU.add)
        psB = ps_s.tile([128, C.DM], F32, name="psB", tag="s",
                        padded_shape=[128, PS_F])
        for (ns, nw) in slices(C.DM, 512):
            nc.tensor.matmul(psB[:, ns:ns + nw], onesb[0:1, :],
                             Rrow[0:1, ns:ns + nw], start=True, stop=True)
        nc.vector.tensor_copy(R_sb[:], psB[:])

    k_proj(0, kT_buf(0), wkh_buf(0))
    q_proj(0, qT_buf(0), wqh_buf(0))

    # ---------------- attention, one head pair at a time ----------------
    carried = {}   # pre-emitted first scores/exp/mask of the next head pair
    for hp in range(C.HP):
        kT = kT_buf(hp)
        qT = qT_buf(hp)
        if hp + 1 < C.HP:
            load_wh(wkh_buf(hp + 1), "k", hp + 1)
            load_wh(wqh_buf(hp + 1), "q", hp + 1)
            nxt_proj = proj_chunks(hp + 1, kT_buf(hp + 1), wkh_buf(hp + 1),
                                   qT_buf(hp + 1), wqh_buf(hp + 1))
        else:
            nxt_proj = []
        for (sq, sw) in slices(C.SQ, C.SQS):
            pv = [
                ps_pv.tile([65, sw], F32, name=f"ps_pv{i}", tag="pv",
                           padded_shape=[65, PS_F])
                for i in range(2)
            ]
            PIPE = 2
            pm_hist = []

            def emit_head(j, i, hp=hp, kT=kT, qT=qT, sq=sq, sw=sw):
                """Scores MMs + exp + mask for one (j, head)."""
                ss = ps_s.tile([128, sw], F32, name=f"ps_sc{i}", tag="s",
                               padded_shape=[128, PS_F])
                for (qs, qw) in slices(sw, 512):
                    nc.tensor.matmul(
                        ss[:, qs:qs + qw],
                        kT[i * 64:(i + 1) * 64, j * 128:(j + 1) * 128],
                        qT[i * 64:(i + 1) * 64, sq + qs: sq + qs + qw],
                        start=True, stop=True,
                    )
                pe = work.tile([128, sw], BF16, name="p_exp", tag="pe",
                               bufs=2, padded_shape=[128, C.SQS])
                nc.scalar.activation(pe[:], ss[:], AF.Exp)
                pm = work.tile([128, sw], BF16, name="p_msk", tag="pm",
                               bufs=5, padded_shape=[128, C.SQS])
                nc.vector.tensor_tensor(
                    out=pm[:], in0=pe[:],
                    in1=maskT_t[j][:, sq: sq + sw],
                    op=ALU.mult,
                )
                return pm

            def emit_pv_half(jj, pmi, i, pv=pv, hp=hp, sw=sw):
                for (qs, qw) in slices(sw, 512):
                    nc.tensor.matmul(
                        pv[i][:, qs:qs + qw], v_view[:, jj, 2 * hp + i, :],
                        pmi[:, qs:qs + qw],
                        start=(jj == 0), stop=(jj == C.NJ - 1),
                    )

            def finalize_head(i, pv=pv, hp=hp, sq=sq, sw=sw):
                # copy PV rows out of PSUM immediately (DVE) and the Z row on
                # ACT, so the pv accumulator slot frees for the next head
                # pair ~3us sooner; normalize from the SBUF copies
                pvn = pv[i][0:64, :]
                zrow = work.tile([1, sw], BF16, name="zrow", tag="zrow",
                                 bufs=1, padded_shape=[1, max(C.DM, C.SQS)])
                nc.scalar.copy(zrow[0:1, :], pv[i][64:65, :])
                zb = ps_s.tile([64, sw], F32, name="zb", tag="s",
                               padded_shape=[128, PS_F])
                for (qs, qw) in slices(sw, 512):
                    nc.tensor.matmul(zb[:, qs:qs + qw], onesb[0:1, 0:64],
                                     zrow[0:1, qs:qs + qw],
                                     start=True, stop=True)
                zr = work.tile([64, sw], F32, name="zr", tag="zr", bufs=1,
                               padded_shape=[64, C.SQS])
                nc.vector.reciprocal_approx_fast(out=zr[:], in_=zb[:])
                tmp = work.tile([64, sw], BF16, name="xat_t", tag="xat_t",
                                bufs=1, padded_shape=[64, C.SQS])
                nc.vector.tensor_tensor(out=tmp[:], in0=pvn,
                                        in1=zr[:], op=ALU.mult)
                nc.sync.dma_start(
                    xattnT[64 * i:64 * (i + 1), hp * C.SQ + sq: hp * C.SQ + sq + sw],
                    tmp[:],
                )

            # software-pipelined emission, PE queue order per iteration:
            # [PV halves (deps long ready), proj chunk, scores j+1] so the
            # head-of-queue never stalls on exp's psum-slot release
            if hp in carried:
                pm_hist.append((0, carried.pop(hp)))
            else:
                pm_hist.append((0, [emit_head(0, 0), emit_head(0, 1)]))
            for j in range(C.NJ):
                if len(pm_hist) >= PIPE:
                    jj, pp = pm_hist.pop(0)
                    emit_pv_half(jj, pp[0], 0)
                    emit_pv_half(jj, pp[1], 1)
                if j == 3 and hp == 1:
                    emit_R()
                if j + 1 < C.NJ:
                    pms_n = [emit_head(j + 1, 0), emit_head(j + 1, 1)]
                    pm_hist.append((j + 1, pms_n))
                # overlap next head pair's projections with this attention;
                # emitted AFTER the scores so ACT's next exp input is never
                # queued behind a 1.7us proj burst
                if j >= 4 and j % 2 == 0 and nxt_proj:
                    nxt_proj.pop(0)()
            # drain: finalize each head right after its last PV half, and
            # pre-emit the next head pair's first scores so ACT's exp
            # stream doesn't idle across the boundary
            for idx, (jj, pp) in enumerate(pm_hist):
                last = idx == len(pm_hist) - 1
                emit_pv_half(jj, pp[0], 0)
                if last:
                    finalize_head(0)
                    if hp + 1 < C.HP:
                        carried[hp + 1] = [emit_head(
                            0, 0, kT=kT_buf(hp + 1), qT=qT_buf(hp + 1))]
                emit_pv_half(jj, pp[1], 1)
                if last:
                    finalize_head(1)
                    if hp + 1 < C.HP:
                        carried[hp + 1].append(emit_head(
                            0, 1, kT=kT_buf(hp + 1), qT=qT_buf(hp + 1)))

    if C.max_stage <= 3:
        for pl in reversed(list(pools.values())):
            pl.release()
        return

    # ---------------- epilogue: output projection ----------------
    for m in range(C.SQ // 128):
        ps = ps_pv.tile([128, C.DM], F32, name="ps_o", tag="pv",
                        padded_shape=[128, PS_F])
        for (qs, qw) in slices(C.DM, 512):
            for hp in range(C.HP):
                nc.tensor.matmul(
                    ps[:, qs:qs + qw],
                    xattnT[:, hp * C.SQ + m * 128: hp * C.SQ + (m + 1) * 128],
                    wo_t[hp][:, qs:qs + qw],
                    start=(hp == 0), stop=(hp == C.HP - 1),
                )
        ot = work.tile([128, C.DM], BF16, name="out_sb", tag="out_sb", bufs=2,
                       padded_shape=[128, PS_F])
        nc.vector.tensor_tensor(out=ot[:], in0=ps[:], in1=R_sb[:], op=ALU.add)
        # SWDGE cast-DMA bf16 -> f32 (Pool is idle in the epilogue)
        nc.gpsimd.dma_start(out[m * 128:(m + 1) * 128, :], ot[:])

    for pl in reversed(list(pools.values())):
        pl.release()


def build(cfg, reps=1):
    nc = bacc.Bacc("TRN2", target_bir_lowering=False, debug=False)
    C = cfg
    io = {
        "xq": nc.dram_tensor("xq", [C.SQ, C.DM], BF16, kind="ExternalInput").ap(),
        "xk": nc.dram_tensor("xk", [C.SK, C.DM], BF16, kind="ExternalInput").ap(),
        "xv": nc.dram_tensor("xv", [C.SK, C.DM], BF16, kind="ExternalInput").ap(),
        "mask": nc.dram_tensor("mask", [C.SQ, C.SK], BF16, kind="ExternalInput").ap(),
        "wqt": nc.dram_tensor("wqt", [C.DM, C.DM], BF16, kind="ExternalInput").ap(),
        "wkt": nc.dram_tensor("wkt", [C.DM, C.DM], BF16, kind="ExternalInput").ap(),
        "wvt": nc.dram_tensor("wvt", [C.DM, C.DM], BF16, kind="ExternalInput").ap(),
        "wot": nc.dram_tensor("wot", [C.DM, C.DM], BF16, kind="ExternalInput").ap(),
        "bql": nc.dram_tensor("bql", [128, C.HP], F32, kind="ExternalInput").ap(),
        "bkl": nc.dram_tensor("bkl", [128, C.HP], F32, kind="ExternalInput").ap(),
        "bvl": nc.dram_tensor("bvl", [128, C.KT], BF16, kind="ExternalInput").ap(),
        "bo_row": nc.dram_tensor("bo_row", [1, C.DM], BF16, kind="ExternalInput").ap(),
        "out": nc.dram_tensor("out", [C.SQ, C.DM], F32, kind="ExternalOutput").ap(),
    }
    with tile.TileContext(nc) as tc:
        for _ in range(reps):
            emit_kernel(tc, cfg, io)
    nc.compile()
    return nc


def host_prep(query, key, value, mask, Wq, bq, Wk, bk, Wv, bv, Wo, bo, cfg):
    """Host-side layout prep (weight transpose/cast, bf16 casts, slicing)."""
    C = cfg
    bf = ml_dtypes.bfloat16
    wqt = np.ascontiguousarray((Wq.T * 0.125).astype(bf))   # 1/sqrt(dk) folded
    wkt = np.ascontiguousarray(Wk.T.astype(bf))
    wvt = np.ascontiguousarray(Wv.T.astype(bf))
    wot = np.ascontiguousarray(Wo.T.astype(bf))
    bql = np.ascontiguousarray((bq * 0.125).reshape(C.HP, 128).T.astype(np.float32))
    bkl = np.ascontiguousarray(bk.reshape(C.HP, 128).T.astype(np.float32))
    bvl = np.ascontiguousarray(bv.reshape(C.KT, 128).T.astype(bf))
    bo_row = np.ascontiguousarray(bo.reshape(1, C.DM).astype(bf))
    shared = dict(wqt=wqt, wkt=wkt, wvt=wvt, wot=wot, bql=bql, bkl=bkl,
                  bvl=bvl, bo_row=bo_row)
    in_maps = []
    B = query.shape[0]
    halves = query.shape[1] // C.SQ
    key_bf = [np.ascontiguousarray(key[b].astype(bf)) for b in range(B)]
    val_bf = [np.ascontiguousarray(value[b].astype(bf)) for b in range(B)]
    for c in range(B * halves):
        b, h = divmod(c, halves)
        m = dict(shared)
        m["xq"] = np.ascontiguousarray(
            query[b, h * C.SQ:(h + 1) * C.SQ, :].astype(bf))
        m["xk"] = key_bf[b]
        m["xv"] = val_bf[b]
        m["mask"] = np.ascontiguousarray(
            mask[b, h * C.SQ:(h + 1) * C.SQ, :].astype(bf))
        in_maps.append(m)
    return in_maps


_CACHED = {}


def get_built():
    if "nc" not in _CACHED:
        _CACHED["nc"] = build(Cfg())
    return _CACHED["nc"]


def kernel(query, key, value, mask, Wq, bq, Wk, bk, Wv, bv, Wo, bo):
    cfg = Cfg()
    nc = get_built()
    in_maps = host_prep(query, key, value, mask, Wq, bq, Wk, bk, Wv, bv, Wo, bo, cfg)
    res = run_bass_kernel_spmd(nc, in_maps, core_ids=list(range(N_CORES)))
    B, S, DM = query.shape
    out = np.empty((B, S, DM), np.float32)
    for c in range(N_CORES):
        b, h = divmod(c, 2)
        out[b, h * cfg.SQ:(h + 1) * cfg.SQ, :] = res.results[c]["out"]
    return out


# revision 64
# speedup vs baseline: 1.2870x; 1.2870x over previous
"""MultiHeadedAttention Trainium2 kernel (8-core SPMD, data-parallel).

Sharding: 8 cores = (batch b in 0..3) x (query half in 0..1). Each core
computes out[b, half*1024:(half+1)*1024, :] independently - no collectives.

v2 pipeline (vs v1: no DRAM staging, per-head-pair proj/attention overlap):
  - host casts x/mask to bf16; device DMA-transposes them straight from the
    input DRAM tensors into SBUF (xbar transpose, 2-byte dtype). All
    transposes on the SP queue only (dual-queue transposes corrupt on HW)
    and phase-separated from DMA copies (Tile serializes copy<->transpose
    pairs ~4us each for the xbar hazard). Every DMA gets its own tile
    (multi-DMA fills of one tile serialize on WAW semaphores).
  - V projection runs first (PE-dense) while k/q/mask transposes stream in
  - per head pair hp: K/Q projection chunks for hp+1 are emitted inside the
    attention j-loop of hp (512-wide psums, emitted after the scores), so PE
    never drains while ACT (exp) is busy
  - attention j-loop is software-pipelined: iteration j emits PV halves of
    j-1 (deps long ready), a proj chunk, then scores/exp/mask for j+1 so
    the in-order PE queue never stalls on exp's psum-slot release
  - scoresT [k,q] psum -> exp on ScalarE -> mask multiply on DVE (bf16 2x
    mode) -> PV accumulate (Z via trailing ones column of v)
  - finalize: Z-row copy on ACT, PE-broadcast Z, reciprocal_approx_fast,
    multiply, DMA hop into xattnT [dm, q]
  - epilogue out = xattnT.T @ WoT + R, R = bv@WoT + bo, bf16 out staging
    cast to f32 by SWDGE during the store
"""
import numpy as np
import ml_dtypes

import concourse.bass as bass
import concourse.mybir as mybir
import concourse.tile as tile
from concourse import bacc
from concourse.bass_utils import run_bass_kernel_spmd

F32 = mybir.dt.float32
BF16 = mybir.dt.bfloat16
FP8 = mybir.dt.float8e4
AF = mybir.ActivationFunctionType
ALU = mybir.AluOpType

N_CORES = 8
DK = 64


def slices(total, chunk):
    return [(s, min(chunk, total - s)) for s in range(0, total, chunk)]


class Cfg:
    def __init__(self, SQ=1024, SK=2048, DM=1024, H=16, max_stage=5):
        assert DM % 128 == 0 and SK % 128 == 0 and SQ % 128 == 0 and H % 2 == 0
        self.SQ, self.SK, self.DM, self.H = SQ, SK, DM, H
        self.KT = DM // 128          # dm contraction chunks
        self.HP = H // 2             # head pairs
        self.NJ = SK // 128          # Sk tiles
        self.SQS = min(1024, SQ)     # attention Sq slice width (2 psum banks)
        self.max_stage = max_stage
        assert SQ % self.SQS == 0
        assert H * DK == DM


def emit_kernel(tc, cfg, io):
    nc = tc.nc
    C = cfg
    xq, xk, xv, msk = io["xq"], io["xk"], io["xv"], io["mask"]
    w_dram = {"q": io["wqt"], "k": io["wkt"], "v": io["wvt"], "o": io["wot"]}
    bqkl, bvl, bo_row = io["bqkl"], io["bvl"], io["bo_row"]
    out = io["out"]
    PS_F = max(C.SQS, 512)

    pools = {}

    def open_pool(name, bufs=1, space="SBUF"):
        pools[name] = tc.alloc_tile_pool(name=name, bufs=bufs, space=space)
        return pools[name]

    persist = open_pool("persist", 1)
    rot = open_pool("rot", 1)
    ps_s = open_pool("ps_s", 2, space="PSUM")
    ps_pv = open_pool("ps_pv", 2, space="PSUM")
    work = open_pool("work", 1)
    poolA = open_pool("A", 1)   # LIFO: A on top so it can release mid-kernel

    # ---------------- persistent tiles ----------------
    # per-DMA-chunk tiles: a multi-DMA fill of ONE tile serializes on Tile's
    # same-tile WAW semaphores (~5us per DMA round trip), so every DMA gets
    # its own tile
    v_sb = persist.tile([128, C.NJ * C.H * 65], BF16, name="v_sb")
    maskT_t = [persist.tile([128, C.SQ], BF16, name=f"maskT{j}")
               for j in range(C.NJ)]
    xkT_t = [persist.tile([128, C.SK], BF16, name=f"xkT{kt}")
             for kt in range(C.KT)]
    xqT_t = [persist.tile([128, C.SQ], BF16, name=f"xqT{kt}")
             for kt in range(C.KT)]
    R_sb = persist.tile([128, C.DM], BF16, name="R_sb")
    bqkl_sb = persist.tile([128, 2 * C.HP], F32, name="bqkl_sb")
    bvl_sb = persist.tile([128, C.KT], BF16, name="bvl_sb")
    bo_sb = persist.tile([1, C.DM], BF16, name="bo_sb")
    onesb = persist.tile([1, 128], BF16, name="onesb")
    onesf = persist.tile([1, 128], F32, name="onesf")

    xvT_t = {(kt, h): poolA.tile([128, C.SK // 2], BF16, name=f"xvT{kt}_{h}")
             for kt in range(C.KT) for h in range(2)}
    wv_sb = poolA.tile([128, C.KT * C.DM], BF16, name="wv_sb")

    v_view = v_sb.rearrange("p (j h c) -> p j h c", j=C.NJ, c=65)

    # rotating double-buffered tiles, memoized so every use of (kind, hp)
    # shares one handle (a second pool.tile() call would alias a NEW tile
    # into the slot instead of reading what the projection wrote)
    _rot_tiles = {}

    def _rot(kind, hp, shape, nbuf=2):
        key = (kind, hp)
        if key not in _rot_tiles:
            _rot_tiles[key] = rot.tile(shape, BF16, name=f"{kind}{hp % nbuf}",
                                       tag=f"{kind}{hp % nbuf}")
        return _rot_tiles[key]

    def kT_buf(hp):
        return _rot("kT", hp, [128, C.SK])

    def qT_buf(hp):
        return _rot("qT", hp, [128, C.SQ])

    # single-buffered: wkh(hp) is fully consumed by proj(hp) during
    # attention(hp-1), before wkh(hp+1)'s load is issued
    def wkh_buf(hp):
        return _rot("wk", hp, [128, C.KT * 128], nbuf=1)

    def wqh_buf(hp):
        return _rot("wq", hp, [128, C.KT * 128], nbuf=1)

    # ---------------- prologue DMAs ----------------
    nc.gpsimd.dma_start(bqkl_sb[:], bqkl[:])
    nc.gpsimd.dma_start(bvl_sb[:], bvl[:])
    nc.gpsimd.dma_start(bo_sb[:], bo_row[:])
    nc.vector.memset(onesb[:], 1.0)
    nc.vector.memset(onesf[:], 1.0)
    nc.vector.memset(v_view[:, :, :, 64:65], 1.0)

    # DMA phase discipline: copies and transposes must not coexist in the
    # schedule window (Tile serializes every DMACopy<->DmaTranspose pair,
    # ~4us each, to dodge a real HW xbar deadlock). All prologue copies
    # first, then all transposes (SP queue only - dual-queue transposes
    # corrupt data on HW).
    def load_wh(dst, name_w, hp):
        # [1024, 128] column slice -> [128, KT*128] (kt-blocked), one DMA
        nc.sync.dma_start(
            dst.rearrange("p (k c) -> p k c", k=C.KT),
            w_dram[name_w][:, hp * 128:(hp + 1) * 128].rearrange(
                "(k p) c -> p k c", p=128),
        )

    # single big DMAs for weights: fewer copy instructions mean the copy
    # phase fully drains before the transposes (each leftover copy would
    # serialize ~6us pairwise against them)
    nc.scalar.dma_start(
        wv_sb.rearrange("p (k c) -> p k c", k=C.KT),
        w_dram["v"].rearrange("(k p) c -> p k c", p=128),
    )
    load_wh(wkh_buf(0), "k", 0)
    load_wh(wqh_buf(0), "q", 0)

    # xvT in Sk-halves (own tile each): V proj's first k-tiles unblock
    # after 8 transposes instead of all 8 full-Sk ones
    for h, (hs, hw) in enumerate(slices(C.SK, C.SK // 2)):
        for kt in range(C.KT):
            nc.sync.dma_start(
                xvT_t[(kt, h)][:],
                xv[hs:hs + hw, kt * 128:(kt + 1) * 128], transpose=True)
    for kt in range(C.KT):
        nc.sync.dma_start(
            xkT_t[kt][:], xk[:, kt * 128:(kt + 1) * 128], transpose=True)
    for kt in range(C.KT):
        nc.sync.dma_start(
            xqT_t[kt][:], xq[:, kt * 128:(kt + 1) * 128], transpose=True)

    # ---------------- V projection (PE-dense pipeline fill) ----------------
    def v_proj_j(j):
        ps = ps_s.tile([128, C.DM], F32, name="ps_v", tag="s",
                       padded_shape=[128, PS_F])
        h, jo = divmod(j * 128, C.SK // 2)
        for (ds_, dw) in slices(C.DM, 512):
            for kt in range(C.KT):
                nc.tensor.matmul(
                    ps[:, ds_:ds_ + dw],
                    xvT_t[(kt, h)][:, jo:jo + 128],
                    wv_sb[:, kt * C.DM + ds_:kt * C.DM + ds_ + dw],
                    start=(kt == 0), stop=(kt == C.KT - 1),
                )
        nc.vector.tensor_copy(
            v_view[:, j, :, 0:64],
            ps.rearrange("p (h c) -> p h c", c=DK),
        )

    for j in range(C.NJ):
        v_proj_j(j)

    # mask: bf16 transposes from DRAM, one per-j tile each (no WAW chain)
    for j in range(C.NJ):
        nc.sync.dma_start(maskT_t[j][:], msk[:, j * 128:(j + 1) * 128],
                          transpose=True)

    poolA.release()
    del pools["A"]
    poolB = open_pool("B", 1)
    xattnT = poolB.tile([128, C.HP * C.SQ], BF16, name="xattnT")
    wo_sb = poolB.tile([128, C.KT * C.DM], BF16, name="wo_sb")
    nc.gpsimd.dma_start(
        wo_sb.rearrange("p (k c) -> p k c", k=C.KT),
        w_dram["o"].rearrange("(k p) c -> p k c", p=128),
    )

    if C.max_stage <= 2:
        for pl in reversed(list(pools.values())):
            pl.release()
        return

    # ---------------- projections (emitted per head pair) ----------------
    # projection psums are 512 wide: short "s"-slot holds so the scores/exp
    # rotation stalls at most ~1.7us when a proj tile steals a slot
    def _proj_chunk(xT_t, w, dst, bias_col, ns, nw):
        ps = ps_s.tile([128, nw], F32, name="ps_kp", tag="s",
                       padded_shape=[128, PS_F])
        for kt in range(C.KT):
            nc.tensor.matmul(
                ps[:],
                w[:, kt * 128:(kt + 1) * 128],
                xT_t[kt][:, ns: ns + nw],
                start=(kt == 0), stop=(kt == C.KT - 1),
            )
        nc.vector.tensor_scalar_add(out=dst[:, ns:ns + nw], in0=ps[:],
                                    scalar1=bias_col)

    def proj_chunks(hp, kT, wk, qT, wq):
        """Closures emitting one 512-wide projection chunk each (4 K + 2 Q)."""
        out = []
        for (ns, nw) in slices(C.SK, 512):
            out.append(lambda ns=ns, nw=nw: _proj_chunk(
                xkT_t, wk, kT, bqkl_sb[:, C.HP + hp:C.HP + hp + 1], ns, nw))
        for (ns, nw) in slices(C.SQ, 512):
            out.append(lambda ns=ns, nw=nw: _proj_chunk(
                xqT_t, wq, qT, bqkl_sb[:, hp:hp + 1], ns, nw))
        return out

    def k_proj(hp, kT, wk):
        for (ns, nw) in slices(C.SK, 512):
            _proj_chunk(xkT_t, wk, kT, bqkl_sb[:, C.HP + hp:C.HP + hp + 1], ns, nw)

    def q_proj(hp, qT, wq):
        for (ns, nw) in slices(C.SQ, 512):
            _proj_chunk(xqT_t, wq, qT, bqkl_sb[:, hp:hp + 1], ns, nw)

    def emit_R():
        # R = bv@WoT + bo, PE-broadcast to 128 rows
        psR = ps_s.tile([1, C.DM], F32, name="psR", tag="s",
                        padded_shape=[128, PS_F])
        for (ns, nw) in slices(C.DM, 512):
            for kt in range(C.KT):
                nc.tensor.matmul(
                    psR[0:1, ns:ns + nw], bvl_sb[:, kt:kt + 1],
                    wo_sb[:, kt * C.DM + ns:kt * C.DM + ns + nw],
                    start=(kt == 0), stop=(kt == C.KT - 1),
                )
        Rrow = work.tile([1, C.DM], BF16, name="Rrow", tag="zrow", bufs=1,
                         padded_shape=[1, max(C.DM, C.SQS)])
        nc.vector.tensor_tensor(out=Rrow[:], in0=psR[:], in1=bo_sb[:],
                                op=ALU.add)
        psB = ps_s.tile([128, C.DM], F32, name="psB", tag="s",
                        padded_shape=[128, PS_F])
        for (ns, nw) in slices(C.DM, 512):
            nc.tensor.matmul(psB[:, ns:ns + nw], onesb[0:1, :],
                             Rrow[0:1, ns:ns + nw], start=True, stop=True)
        nc.vector.tensor_copy(R_sb[:], psB[:])

    k_proj(0, kT_buf(0), wkh_buf(0))
    q_proj(0, qT_buf(0), wqh_buf(0))

    # ---------------- attention, one head pair at a time ----------------
    carried = {}   # pre-emitted first scores/exp/mask of the next head pair
    for hp in range(C.HP):
        kT = kT_buf(hp)
        qT = qT_buf(hp)
        if hp + 1 < C.HP:
            load_wh(wkh_buf(hp + 1), "k", hp + 1)
            load_wh(wqh_buf(hp + 1), "q", hp + 1)
            nxt_proj = proj_chunks(hp + 1, kT_buf(hp + 1), wkh_buf(hp + 1),
                                   qT_buf(hp + 1), wqh_buf(hp + 1))
        else:
            nxt_proj = []
        for (sq, sw) in slices(C.SQ, C.SQS):
            pv = [
                ps_pv.tile([65, sw], F32, name=f"ps_pv{i}", tag="pv",
                           padded_shape=[65, PS_F])
                for i in range(2)
            ]
            PIPE = 2
            pm_hist = []

            def emit_head(j, i, hp=hp, kT=kT, qT=qT, sq=sq, sw=sw):
                """Scores MMs + exp + mask for one (j, head)."""
                ss = ps_s.tile([128, sw], F32, name=f"ps_sc{i}", tag="s",
                               padded_shape=[128, PS_F])
                for (qs, qw) in slices(sw, 512):
                    nc.tensor.matmul(
                        ss[:, qs:qs + qw],
                        kT[i * 64:(i + 1) * 64, j * 128:(j + 1) * 128],
                        qT[i * 64:(i + 1) * 64, sq + qs: sq + qs + qw],
                        start=True, stop=True,
                    )
                pe = work.tile([128, sw], BF16, name="p_exp", tag="pe",
                               bufs=2, padded_shape=[128, C.SQS])
                nc.scalar.activation(pe[:], ss[:], AF.Exp)
                pm = work.tile([128, sw], BF16, name="p_msk", tag="pm",
                               bufs=5, padded_shape=[128, C.SQS])
                nc.vector.tensor_tensor(
                    out=pm[:], in0=pe[:],
                    in1=maskT_t[j][:, sq: sq + sw],
                    op=ALU.mult,
                )
                return pm

            def emit_pv_half(jj, pmi, i, pv=pv, hp=hp, sw=sw):
                for (qs, qw) in slices(sw, 512):
                    nc.tensor.matmul(
                        pv[i][:, qs:qs + qw], v_view[:, jj, 2 * hp + i, :],
                        pmi[:, qs:qs + qw],
                        start=(jj == 0), stop=(jj == C.NJ - 1),
                    )

            def finalize_head(i, pv=pv, hp=hp, sq=sq, sw=sw):
                # copy PV rows out of PSUM immediately (DVE) and the Z row on
                # ACT, so the pv accumulator slot frees for the next head
                # pair ~3us sooner; normalize from the SBUF copies
                pvn = pv[i][0:64, :]
                zrow = work.tile([1, sw], BF16, name="zrow", tag="zrow",
                                 bufs=1, padded_shape=[1, max(C.DM, C.SQS)])
                nc.scalar.copy(zrow[0:1, :], pv[i][64:65, :])
                zb = ps_s.tile([64, sw], F32, name="zb", tag="s",
                               padded_shape=[128, PS_F])
                for (qs, qw) in slices(sw, 512):
                    nc.tensor.matmul(zb[:, qs:qs + qw], onesb[0:1, 0:64],
                                     zrow[0:1, qs:qs + qw],
                                     start=True, stop=True)
                zr = work.tile([64, sw], F32, name="zr", tag="zr", bufs=1,
                               padded_shape=[64, C.SQS])
                nc.vector.reciprocal_approx_fast(out=zr[:], in_=zb[:])
                tmp = work.tile([64, sw], BF16, name="xat_t", tag="xat_t",
                                bufs=1, padded_shape=[64, C.SQS])
                nc.vector.tensor_tensor(out=tmp[:], in0=pvn,
                                        in1=zr[:], op=ALU.mult)
                nc.sync.dma_start(
                    xattnT[64 * i:64 * (i + 1), hp * C.SQ + sq: hp * C.SQ + sq + sw],
                    tmp[:],
                )

            # software-pipelined emission, PE queue order per iteration:
            # [PV halves (deps long ready), proj chunk, scores j+1] so the
            # head-of-queue never stalls on exp's psum-slot release
            if hp in carried:
                pm_hist.append((0, carried.pop(hp)))
            else:
                pm_hist.append((0, [emit_head(0, 0), emit_head(0, 1)]))
            for j in range(C.NJ):
                if len(pm_hist) >= PIPE:
                    jj, pp = pm_hist.pop(0)
                    emit_pv_half(jj, pp[0], 0)
                    emit_pv_half(jj, pp[1], 1)
                if j == 3 and hp == 1:
                    emit_R()
                if j + 1 < C.NJ:
                    pms_n = [emit_head(j + 1, 0), emit_head(j + 1, 1)]
                    pm_hist.append((j + 1, pms_n))
                # overlap next head pair's projections with this attention;
                # emitted AFTER the scores so ACT's next exp input is never
                # queued behind a 1.7us proj burst
                if j >= 4 and j % 2 == 0 and nxt_proj:
                    nxt_proj.pop(0)()
            # drain: finalize each head right after its last PV half, and
            # pre-emit the next head pair's first scores so ACT's exp
            # stream doesn't idle across the boundary
            for idx, (jj, pp) in enumerate(pm_hist):
                last = idx == len(pm_hist) - 1
                emit_pv_half(jj, pp[0], 0)
                if last:
                    finalize_head(0)
                    if hp + 1 < C.HP:
                        carried[hp + 1] = [emit_head(
                            0, 0, kT=kT_buf(hp + 1), qT=qT_buf(hp + 1))]
                emit_pv_half(jj, pp[1], 1)
                if last:
                    finalize_head(1)
                    if hp + 1 < C.HP:
                        carried[hp + 1].append(emit_head(
                            0, 1, kT=kT_buf(hp + 1), qT=qT_buf(hp + 1)))

    if C.max_stage <= 3:
        for pl in reversed(list(pools.values())):
            pl.release()
        return

    # ---------------- epilogue: output projection ----------------
    for m in range(C.SQ // 128):
        ps = ps_pv.tile([128, C.DM], F32, name="ps_o", tag="pv",
                        padded_shape=[128, PS_F])
        for (qs, qw) in slices(C.DM, 512):
            for hp in range(C.HP):
                nc.tensor.matmul(
                    ps[:, qs:qs + qw],
                    xattnT[:, hp * C.SQ + m * 128: hp * C.SQ + (m + 1) * 128],
                    wo_sb[:, hp * C.DM + qs:hp * C.DM + qs + qw],
                    start=(hp == 0), stop=(hp == C.HP - 1),
                )
        ot = work.tile([128, C.DM], BF16, name="out_sb", tag="out_sb", bufs=2,
                       padded_shape=[128, PS_F])
        nc.vector.tensor_tensor(out=ot[:], in0=ps[:], in1=R_sb[:], op=ALU.add)
        # SWDGE cast-DMA bf16 -> f32 (Pool is idle in the epilogue)
        nc.gpsimd.dma_start(out[m * 128:(m + 1) * 128, :], ot[:])

    for pl in reversed(list(pools.values())):
        pl.release()


def build(cfg, reps=1):
    nc = bacc.Bacc("TRN2", target_bir_lowering=False, debug=False)
    C = cfg
    io = {
        "xq": nc.dram_tensor("xq", [C.SQ, C.DM], BF16, kind="ExternalInput").ap(),
        "xk": nc.dram_tensor("xk", [C.SK, C.DM], BF16, kind="ExternalInput").ap(),
        "xv": nc.dram_tensor("xv", [C.SK, C.DM], BF16, kind="ExternalInput").ap(),
        "mask": nc.dram_tensor("mask", [C.SQ, C.SK], BF16, kind="ExternalInput").ap(),
        "wqt": nc.dram_tensor("wqt", [C.DM, C.DM], BF16, kind="ExternalInput").ap(),
        "wkt": nc.dram_tensor("wkt", [C.DM, C.DM], BF16, kind="ExternalInput").ap(),
        "wvt": nc.dram_tensor("wvt", [C.DM, C.DM], BF16, kind="ExternalInput").ap(),
        "wot": nc.dram_tensor("wot", [C.DM, C.DM], BF16, kind="ExternalInput").ap(),
        "bqkl": nc.dram_tensor("bqkl", [128, 2 * C.HP], F32, kind="ExternalInput").ap(),
        "bvl": nc.dram_tensor("bvl", [128, C.KT], BF16, kind="ExternalInput").ap(),
        "bo_row": nc.dram_tensor("bo_row", [1, C.DM], BF16, kind="ExternalInput").ap(),
        "out": nc.dram_tensor("out", [C.SQ, C.DM], F32, kind="ExternalOutput").ap(),
    }
    with tile.TileContext(nc) as tc:
        for _ in range(reps):
            emit_kernel(tc, cfg, io)
    nc.compile()
    return nc


def host_prep(query, key, value, mask, Wq, bq, Wk, bk, Wv, bv, Wo, bo, cfg):
    """Host-side layout prep (weight transpose/cast, bf16 casts, slicing)."""
    C = cfg
    bf = ml_dtypes.bfloat16
    wqt = np.ascontiguousarray((Wq.T * 0.125).astype(bf))   # 1/sqrt(dk) folded
    wkt = np.ascontiguousarray(Wk.T.astype(bf))
    wvt = np.ascontiguousarray(Wv.T.astype(bf))
    wot = np.ascontiguousarray(Wo.T.astype(bf))
    bql = (bq * 0.125).reshape(C.HP, 128).T.astype(np.float32)
    bkl = bk.reshape(C.HP, 128).T.astype(np.float32)
    bqkl = np.ascontiguousarray(np.concatenate([bql, bkl], axis=1))
    bvl = np.ascontiguousarray(bv.reshape(C.KT, 128).T.astype(bf))
    bo_row = np.ascontiguousarray(bo.reshape(1, C.DM).astype(bf))
    shared = dict(wqt=wqt, wkt=wkt, wvt=wvt, wot=wot, bqkl=bqkl,
                  bvl=bvl, bo_row=bo_row)
    in_maps = []
    B = query.shape[0]
    halves = query.shape[1] // C.SQ
    key_bf = [np.ascontiguousarray(key[b].astype(bf)) for b in range(B)]
    val_bf = [np.ascontiguousarray(value[b].astype(bf)) for b in range(B)]
    for c in range(B * halves):
        b, h = divmod(c, halves)
        m = dict(shared)
        m["xq"] = np.ascontiguousarray(
            query[b, h * C.SQ:(h + 1) * C.SQ, :].astype(bf))
        m["xk"] = key_bf[b]
        m["xv"] = val_bf[b]
        m["mask"] = np.ascontiguousarray(
            mask[b, h * C.SQ:(h + 1) * C.SQ, :].astype(bf))
        in_maps.append(m)
    return in_maps


_CACHED = {}


def get_built():
    if "nc" not in _CACHED:
        _CACHED["nc"] = build(Cfg())
    return _CACHED["nc"]


def kernel(query, key, value, mask, Wq, bq, Wk, bk, Wv, bv, Wo, bo):
    cfg = Cfg()
    nc = get_built()
    in_maps = host_prep(query, key, value, mask, Wq, bq, Wk, bk, Wv, bv, Wo, bo, cfg)
    res = run_bass_kernel_spmd(nc, in_maps, core_ids=list(range(N_CORES)))
    B, S, DM = query.shape
    out = np.empty((B, S, DM), np.float32)
    for c in range(N_CORES):
        b, h = divmod(c, 2)
        out[b, h * cfg.SQ:(h + 1) * cfg.SQ, :] = res.results[c]["out"]
    return out


# revision 66
# speedup vs baseline: 1.3352x; 1.0375x over previous
"""MultiHeadedAttention Trainium2 kernel (8-core SPMD, data-parallel).

Sharding: 8 cores = (batch b in 0..3) x (query half in 0..1). Each core
computes out[b, half*1024:(half+1)*1024, :] independently - no collectives.

v2 pipeline (vs v1: no DRAM staging, per-head-pair proj/attention overlap):
  - host casts x/mask to bf16; device DMA-transposes them straight from the
    input DRAM tensors into SBUF (xbar transpose, 2-byte dtype). All
    transposes on the SP queue only (dual-queue transposes corrupt on HW)
    and phase-separated from DMA copies (Tile serializes copy<->transpose
    pairs ~4us each for the xbar hazard). Every DMA gets its own tile
    (multi-DMA fills of one tile serialize on WAW semaphores).
  - V projection runs first (PE-dense) while k/q/mask transposes stream in
  - per head pair hp: K/Q projection chunks for hp+1 are emitted inside the
    attention j-loop of hp (512-wide psums, emitted after the scores), so PE
    never drains while ACT (exp) is busy
  - attention j-loop is software-pipelined: iteration j emits PV halves of
    j-1 (deps long ready), a proj chunk, then scores/exp/mask for j+1 so
    the in-order PE queue never stalls on exp's psum-slot release
  - scoresT [k,q] psum -> exp on ScalarE -> mask multiply on DVE (bf16 2x
    mode) -> PV accumulate (Z via trailing ones column of v)
  - finalize: Z-row copy on ACT, PE-broadcast Z, reciprocal_approx_fast,
    multiply, DMA hop into xattnT [dm, q]
  - epilogue out = xattnT.T @ WoT + R, R = bv@WoT + bo, bf16 out staging
    cast to f32 by SWDGE during the store
"""
import numpy as np
import ml_dtypes

import concourse.bass as bass
import concourse.mybir as mybir
import concourse.tile as tile
from concourse import bacc
from concourse.bass_utils import run_bass_kernel_spmd

F32 = mybir.dt.float32
BF16 = mybir.dt.bfloat16
FP8 = mybir.dt.float8e4
AF = mybir.ActivationFunctionType
ALU = mybir.AluOpType

N_CORES = 8
DK = 64


def slices(total, chunk):
    return [(s, min(chunk, total - s)) for s in range(0, total, chunk)]


class Cfg:
    def __init__(self, SQ=1024, SK=2048, DM=1024, H=16, max_stage=5):
        assert DM % 128 == 0 and SK % 128 == 0 and SQ % 128 == 0 and H % 2 == 0
        self.SQ, self.SK, self.DM, self.H = SQ, SK, DM, H
        self.KT = DM // 128          # dm contraction chunks
        self.HP = H // 2             # head pairs
        self.NJ = SK // 128          # Sk tiles
        self.SQS = min(1024, SQ)     # attention Sq slice width (2 psum banks)
        self.max_stage = max_stage
        assert SQ % self.SQS == 0
        assert H * DK == DM


def emit_kernel(tc, cfg, io):
    nc = tc.nc
    C = cfg
    xq, xk, xv, msk = io["xq"], io["xk"], io["xv"], io["mask"]
    w_dram = {"q": io["wqt"], "k": io["wkt"], "v": io["wvt"], "o": io["wot"]}
    bqkl, bvl, bo_row = io["bqkl"], io["bvl"], io["bo_row"]
    out = io["out"]
    PS_F = max(C.SQS, 512)

    pools = {}

    def open_pool(name, bufs=1, space="SBUF"):
        pools[name] = tc.alloc_tile_pool(name=name, bufs=bufs, space=space)
        return pools[name]

    persist = open_pool("persist", 1)
    rot = open_pool("rot", 1)
    ps_s = open_pool("ps_s", 2, space="PSUM")
    ps_pv = open_pool("ps_pv", 2, space="PSUM")
    work = open_pool("work", 1)
    poolA = open_pool("A", 1)   # LIFO: A on top so it can release mid-kernel

    # ---------------- persistent tiles ----------------
    # per-DMA-chunk tiles: a multi-DMA fill of ONE tile serializes on Tile's
    # same-tile WAW semaphores (~5us per DMA round trip), so every DMA gets
    # its own tile
    v_sb = persist.tile([128, C.NJ * C.H * 65], BF16, name="v_sb")
    maskT_t = [persist.tile([128, C.SQ], BF16, name=f"maskT{j}")
               for j in range(C.NJ)]
    xkT_t = [persist.tile([128, C.SK], BF16, name=f"xkT{kt}")
             for kt in range(C.KT)]
    xqT_t = [persist.tile([128, C.SQ], BF16, name=f"xqT{kt}")
             for kt in range(C.KT)]
    R_sb = persist.tile([128, C.DM], BF16, name="R_sb")
    bqkl_sb = persist.tile([128, 2 * C.HP], F32, name="bqkl_sb")
    bvl_sb = persist.tile([128, C.KT], BF16, name="bvl_sb")
    bo_sb = persist.tile([1, C.DM], BF16, name="bo_sb")
    onesb = persist.tile([1, 128], BF16, name="onesb")
    onesf = persist.tile([1, 128], F32, name="onesf")

    xvT_t = {(kt, h): poolA.tile([128, C.SK // 2], BF16, name=f"xvT{kt}_{h}")
             for kt in range(C.KT) for h in range(2)}
    wv_sb = poolA.tile([128, C.KT * C.DM], BF16, name="wv_sb")

    v_view = v_sb.rearrange("p (j h c) -> p j h c", j=C.NJ, c=65)

    # rotating double-buffered tiles, memoized so every use of (kind, hp)
    # shares one handle (a second pool.tile() call would alias a NEW tile
    # into the slot instead of reading what the projection wrote)
    _rot_tiles = {}

    def _rot(kind, hp, shape, nbuf=2):
        key = (kind, hp)
        if key not in _rot_tiles:
            _rot_tiles[key] = rot.tile(shape, BF16, name=f"{kind}{hp % nbuf}",
                                       tag=f"{kind}{hp % nbuf}")
        return _rot_tiles[key]

    def kT_buf(hp):
        return _rot("kT", hp, [128, C.SK])

    def qT_buf(hp):
        return _rot("qT", hp, [128, C.SQ])

    # single-buffered: wkh(hp) is fully consumed by proj(hp) during
    # attention(hp-1), before wkh(hp+1)'s load is issued
    def wkh_buf(hp):
        return _rot("wk", hp, [128, C.KT * 128], nbuf=1)

    def wqh_buf(hp):
        return _rot("wq", hp, [128, C.KT * 128], nbuf=1)

    # ---------------- prologue DMAs ----------------
    nc.gpsimd.dma_start(bqkl_sb[:], bqkl[:])
    nc.gpsimd.dma_start(bvl_sb[:], bvl[:])
    nc.gpsimd.dma_start(bo_sb[:], bo_row[:])
    nc.vector.memset(onesb[:], 1.0)
    nc.vector.memset(onesf[:], 1.0)
    nc.vector.memset(v_view[:, :, :, 64:65], 1.0)

    # DMA phase discipline: copies and transposes must not coexist in the
    # schedule window (Tile serializes every DMACopy<->DmaTranspose pair,
    # ~4us each, to dodge a real HW xbar deadlock). All prologue copies
    # first, then all transposes (SP queue only - dual-queue transposes
    # corrupt data on HW).
    def load_wh(dst, name_w, hp):
        # [1024, 128] column slice -> [128, KT*128] (kt-blocked), one DMA
        nc.sync.dma_start(
            dst.rearrange("p (k c) -> p k c", k=C.KT),
            w_dram[name_w][:, hp * 128:(hp + 1) * 128].rearrange(
                "(k p) c -> p k c", p=128),
        )

    # single big DMAs for weights: fewer copy instructions mean the copy
    # phase fully drains before the transposes (each leftover copy would
    # serialize ~6us pairwise against them)
    nc.scalar.dma_start(
        wv_sb.rearrange("p (k c) -> p k c", k=C.KT),
        w_dram["v"].rearrange("(k p) c -> p k c", p=128),
    )
    load_wh(wkh_buf(0), "k", 0)
    load_wh(wqh_buf(0), "q", 0)

    # xvT in Sk-halves (own tile each): V proj's first k-tiles unblock
    # after 8 transposes instead of all 8 full-Sk ones
    for h, (hs, hw) in enumerate(slices(C.SK, C.SK // 2)):
        for kt in range(C.KT):
            nc.sync.dma_start(
                xvT_t[(kt, h)][:],
                xv[hs:hs + hw, kt * 128:(kt + 1) * 128], transpose=True)
    for kt in range(C.KT):
        nc.sync.dma_start(
            xkT_t[kt][:], xk[:, kt * 128:(kt + 1) * 128], transpose=True)
    for kt in range(C.KT):
        nc.sync.dma_start(
            xqT_t[kt][:], xq[:, kt * 128:(kt + 1) * 128], transpose=True)

    # ---------------- V projection (PE-dense pipeline fill) ----------------
    def v_proj_j(j):
        ps = ps_s.tile([128, C.DM], F32, name="ps_v", tag="s",
                       padded_shape=[128, PS_F])
        h, jo = divmod(j * 128, C.SK // 2)
        for (ds_, dw) in slices(C.DM, 512):
            for kt in range(C.KT):
                nc.tensor.matmul(
                    ps[:, ds_:ds_ + dw],
                    xvT_t[(kt, h)][:, jo:jo + 128],
                    wv_sb[:, kt * C.DM + ds_:kt * C.DM + ds_ + dw],
                    start=(kt == 0), stop=(kt == C.KT - 1),
                )
        nc.vector.tensor_copy(
            v_view[:, j, :, 0:64],
            ps.rearrange("p (h c) -> p h c", c=DK),
        )

    for j in range(C.NJ):
        v_proj_j(j)

    # mask: bf16 transposes from DRAM, one per-j tile each (no WAW chain)
    for j in range(C.NJ):
        nc.sync.dma_start(maskT_t[j][:], msk[:, j * 128:(j + 1) * 128],
                          transpose=True)

    poolA.release()
    del pools["A"]
    poolB = open_pool("B", 1)
    xattnT = poolB.tile([128, C.HP * C.SQ], BF16, name="xattnT")
    wo_sb = poolB.tile([128, C.KT * C.DM], BF16, name="wo_sb")
    nc.gpsimd.dma_start(
        wo_sb.rearrange("p (k c) -> p k c", k=C.KT),
        w_dram["o"].rearrange("(k p) c -> p k c", p=128),
    )

    if C.max_stage <= 2:
        for pl in reversed(list(pools.values())):
            pl.release()
        return

    # ---------------- projections (emitted per head pair) ----------------
    # projection psums are 512 wide: short "s"-slot holds so the scores/exp
    # rotation stalls at most ~1.7us when a proj tile steals a slot
    def _proj_chunk(xT_t, w, dst, bias_col, ns, nw):
        ps = ps_s.tile([128, nw], F32, name="ps_kp", tag="s",
                       padded_shape=[128, PS_F])
        for kt in range(C.KT):
            nc.tensor.matmul(
                ps[:],
                w[:, kt * 128:(kt + 1) * 128],
                xT_t[kt][:, ns: ns + nw],
                start=(kt == 0), stop=(kt == C.KT - 1),
            )
        nc.vector.tensor_scalar_add(out=dst[:, ns:ns + nw], in0=ps[:],
                                    scalar1=bias_col)

    def proj_chunks(hp, kT, wk, qT, wq):
        """Closures emitting one 512-wide projection chunk each (4 K + 2 Q)."""
        out = []
        for (ns, nw) in slices(C.SK, 512):
            out.append(lambda ns=ns, nw=nw: _proj_chunk(
                xkT_t, wk, kT, bqkl_sb[:, C.HP + hp:C.HP + hp + 1], ns, nw))
        for (ns, nw) in slices(C.SQ, 512):
            out.append(lambda ns=ns, nw=nw: _proj_chunk(
                xqT_t, wq, qT, bqkl_sb[:, hp:hp + 1], ns, nw))
        return out

    def k_proj(hp, kT, wk):
        for (ns, nw) in slices(C.SK, 512):
            _proj_chunk(xkT_t, wk, kT, bqkl_sb[:, C.HP + hp:C.HP + hp + 1], ns, nw)

    def q_proj(hp, qT, wq):
        for (ns, nw) in slices(C.SQ, 512):
            _proj_chunk(xqT_t, wq, qT, bqkl_sb[:, hp:hp + 1], ns, nw)

    def emit_R():
        # R = bv@WoT + bo, PE-broadcast to 128 rows
        psR = ps_s.tile([1, C.DM], F32, name="psR", tag="s",
                        padded_shape=[128, PS_F])
        for (ns, nw) in slices(C.DM, 512):
            for kt in range(C.KT):
                nc.tensor.matmul(
                    psR[0:1, ns:ns + nw], bvl_sb[:, kt:kt + 1],
                    wo_sb[:, kt * C.DM + ns:kt * C.DM + ns + nw],
                    start=(kt == 0), stop=(kt == C.KT - 1),
                )
        Rrow = work.tile([1, C.DM], BF16, name="Rrow", tag="zrow", bufs=1,
                         padded_shape=[1, max(C.DM, C.SQS)])
        nc.vector.tensor_tensor(out=Rrow[:], in0=psR[:], in1=bo_sb[:],
                                op=ALU.add)
        psB = ps_s.tile([128, C.DM], F32, name="psB", tag="s",
                        padded_shape=[128, PS_F])
        for (ns, nw) in slices(C.DM, 512):
            nc.tensor.matmul(psB[:, ns:ns + nw], onesb[0:1, :],
                             Rrow[0:1, ns:ns + nw], start=True, stop=True)
        nc.vector.tensor_copy(R_sb[:], psB[:])

    k_proj(0, kT_buf(0), wkh_buf(0))
    q_proj(0, qT_buf(0), wqh_buf(0))

    # ---------------- attention, one head pair at a time ----------------
    carried = {}   # pre-emitted first scores/exp/mask of the next head pair
    for hp in range(C.HP):
        kT = kT_buf(hp)
        qT = qT_buf(hp)
        if hp + 1 < C.HP:
            load_wh(wkh_buf(hp + 1), "k", hp + 1)
            load_wh(wqh_buf(hp + 1), "q", hp + 1)
            nxt_proj = proj_chunks(hp + 1, kT_buf(hp + 1), wkh_buf(hp + 1),
                                   qT_buf(hp + 1), wqh_buf(hp + 1))
        else:
            nxt_proj = []
        for (sq, sw) in slices(C.SQ, C.SQS):
            pv = [
                ps_pv.tile([65, sw], F32, name=f"ps_pv{i}", tag="pv",
                           padded_shape=[65, PS_F])
                for i in range(2)
            ]
            PIPE = 2
            pm_hist = []

            def emit_head(j, i, hp=hp, kT=kT, qT=qT, sq=sq, sw=sw):
                """Scores MMs + exp + mask for one (j, head)."""
                ss = ps_s.tile([128, sw], F32, name=f"ps_sc{i}", tag="s",
                               padded_shape=[128, PS_F])
                for (qs, qw) in slices(sw, 512):
                    nc.tensor.matmul(
                        ss[:, qs:qs + qw],
                        kT[i * 64:(i + 1) * 64, j * 128:(j + 1) * 128],
                        qT[i * 64:(i + 1) * 64, sq + qs: sq + qs + qw],
                        start=True, stop=True,
                    )
                pe = work.tile([128, sw], BF16, name="p_exp", tag="pe",
                               bufs=2, padded_shape=[128, C.SQS])
                nc.scalar.activation(pe[:], ss[:], AF.Exp)
                pm = work.tile([128, sw], BF16, name="p_msk", tag="pm",
                               bufs=5, padded_shape=[128, C.SQS])
                nc.vector.tensor_tensor(
                    out=pm[:], in0=pe[:],
                    in1=maskT_t[j][:, sq: sq + sw],
                    op=ALU.mult,
                )
                return pm

            def emit_pv_half(jj, pmi, i, pv=pv, hp=hp, sw=sw):
                for (qs, qw) in slices(sw, 512):
                    nc.tensor.matmul(
                        pv[i][:, qs:qs + qw], v_view[:, jj, 2 * hp + i, :],
                        pmi[:, qs:qs + qw],
                        start=(jj == 0), stop=(jj == C.NJ - 1),
                    )

            def finalize_head(i, pv=pv, hp=hp, sq=sq, sw=sw):
                # copy PV rows out of PSUM immediately (DVE) and the Z row on
                # ACT, so the pv accumulator slot frees for the next head
                # pair ~3us sooner; normalize from the SBUF copies
                pvn = pv[i][0:64, :]
                zrow = work.tile([1, sw], BF16, name="zrow", tag="zrow",
                                 bufs=1, padded_shape=[1, max(C.DM, C.SQS)])
                nc.scalar.copy(zrow[0:1, :], pv[i][64:65, :])
                zb = ps_s.tile([64, sw], F32, name="zb", tag="s",
                               padded_shape=[128, PS_F])
                for (qs, qw) in slices(sw, 512):
                    nc.tensor.matmul(zb[:, qs:qs + qw], onesb[0:1, 0:64],
                                     zrow[0:1, qs:qs + qw],
                                     start=True, stop=True)
                zr = work.tile([64, sw], F32, name="zr", tag="zr", bufs=1,
                               padded_shape=[64, C.SQS])
                nc.vector.reciprocal_approx_fast(out=zr[:], in_=zb[:])
                tmp = work.tile([64, sw], BF16, name="xat_t", tag="xat_t",
                                bufs=1, padded_shape=[64, C.SQS])
                nc.vector.tensor_tensor(out=tmp[:], in0=pvn,
                                        in1=zr[:], op=ALU.mult)
                nc.sync.dma_start(
                    xattnT[64 * i:64 * (i + 1), hp * C.SQ + sq: hp * C.SQ + sq + sw],
                    tmp[:],
                )

            # software-pipelined emission, PE queue order per iteration:
            # [PV halves (deps long ready), proj chunk, scores j+1] so the
            # head-of-queue never stalls on exp's psum-slot release
            if hp in carried:
                pm_hist.append((0, carried.pop(hp)))
            else:
                pm_hist.append((0, [emit_head(0, 0), emit_head(0, 1)]))
            for j in range(C.NJ):
                if len(pm_hist) >= PIPE:
                    jj, pp = pm_hist.pop(0)
                    emit_pv_half(jj, pp[0], 0)
                    emit_pv_half(jj, pp[1], 1)
                if j == 3 and hp == 1:
                    emit_R()
                if j + 1 < C.NJ:
                    pms_n = [emit_head(j + 1, 0), emit_head(j + 1, 1)]
                    pm_hist.append((j + 1, pms_n))
                # overlap next head pair's projections with this attention;
                # emitted AFTER the scores so ACT's next exp input is never
                # queued behind a 1.7us proj burst
                if j >= 4 and j % 2 == 0 and nxt_proj:
                    nxt_proj.pop(0)()
            # drain: finalize each head right after its last PV half, and
            # pre-emit the next head pair's first scores so ACT's exp
            # stream doesn't idle across the boundary
            for idx, (jj, pp) in enumerate(pm_hist):
                last = idx == len(pm_hist) - 1
                emit_pv_half(jj, pp[0], 0)
                if last:
                    finalize_head(0)
                    if hp + 1 < C.HP:
                        carried[hp + 1] = [emit_head(
                            0, 0, kT=kT_buf(hp + 1), qT=qT_buf(hp + 1))]
                emit_pv_half(jj, pp[1], 1)
                if last:
                    finalize_head(1)
                    if hp + 1 < C.HP:
                        carried[hp + 1].append(emit_head(
                            0, 1, kT=kT_buf(hp + 1), qT=qT_buf(hp + 1)))

    if C.max_stage <= 3:
        for pl in reversed(list(pools.values())):
            pl.release()
        return

    # ---------------- epilogue: output projection ----------------
    for m in range(C.SQ // 128):
        # alternate "s"/"pv" slots: 4-deep psum rotation, and the first
        # tiles use the free "s" banks instead of waiting on hp7's pv
        ps = (ps_s if m % 2 == 0 else ps_pv).tile(
            [128, C.DM], F32, name="ps_o", tag="s" if m % 2 == 0 else "pv",
            padded_shape=[128, PS_F])
        for (qs, qw) in slices(C.DM, 512):
            for hp in range(C.HP):
                nc.tensor.matmul(
                    ps[:, qs:qs + qw],
                    xattnT[:, hp * C.SQ + m * 128: hp * C.SQ + (m + 1) * 128],
                    wo_sb[:, hp * C.DM + qs:hp * C.DM + qs + qw],
                    start=(hp == 0), stop=(hp == C.HP - 1),
                )
        ot = work.tile([128, C.DM], BF16, name="out_sb", tag="out_sb", bufs=2,
                       padded_shape=[128, PS_F])
        nc.vector.tensor_tensor(out=ot[:], in0=ps[:], in1=R_sb[:], op=ALU.add)
        # SWDGE cast-DMA bf16 -> f32 (Pool is idle in the epilogue)
        nc.gpsimd.dma_start(out[m * 128:(m + 1) * 128, :], ot[:])

    for pl in reversed(list(pools.values())):
        pl.release()


def build(cfg, reps=1):
    nc = bacc.Bacc("TRN2", target_bir_lowering=False, debug=False)
    C = cfg
    io = {
        "xq": nc.dram_tensor("xq", [C.SQ, C.DM], BF16, kind="ExternalInput").ap(),
        "xk": nc.dram_tensor("xk", [C.SK, C.DM], BF16, kind="ExternalInput").ap(),
        "xv": nc.dram_tensor("xv", [C.SK, C.DM], BF16, kind="ExternalInput").ap(),
        "mask": nc.dram_tensor("mask", [C.SQ, C.SK], BF16, kind="ExternalInput").ap(),
        "wqt": nc.dram_tensor("wqt", [C.DM, C.DM], BF16, kind="ExternalInput").ap(),
        "wkt": nc.dram_tensor("wkt", [C.DM, C.DM], BF16, kind="ExternalInput").ap(),
        "wvt": nc.dram_tensor("wvt", [C.DM, C.DM], BF16, kind="ExternalInput").ap(),
        "wot": nc.dram_tensor("wot", [C.DM, C.DM], BF16, kind="ExternalInput").ap(),
        "bqkl": nc.dram_tensor("bqkl", [128, 2 * C.HP], F32, kind="ExternalInput").ap(),
        "bvl": nc.dram_tensor("bvl", [128, C.KT], BF16, kind="ExternalInput").ap(),
        "bo_row": nc.dram_tensor("bo_row", [1, C.DM], BF16, kind="ExternalInput").ap(),
        "out": nc.dram_tensor("out", [C.SQ, C.DM], F32, kind="ExternalOutput").ap(),
    }
    with tile.TileContext(nc) as tc:
        for _ in range(reps):
            emit_kernel(tc, cfg, io)
    nc.compile()
    return nc


def host_prep(query, key, value, mask, Wq, bq, Wk, bk, Wv, bv, Wo, bo, cfg):
    """Host-side layout prep (weight transpose/cast, bf16 casts, slicing)."""
    C = cfg
    bf = ml_dtypes.bfloat16
    wqt = np.ascontiguousarray((Wq.T * 0.125).astype(bf))   # 1/sqrt(dk) folded
    wkt = np.ascontiguousarray(Wk.T.astype(bf))
    wvt = np.ascontiguousarray(Wv.T.astype(bf))
    wot = np.ascontiguousarray(Wo.T.astype(bf))
    bql = (bq * 0.125).reshape(C.HP, 128).T.astype(np.float32)
    bkl = bk.reshape(C.HP, 128).T.astype(np.float32)
    bqkl = np.ascontiguousarray(np.concatenate([bql, bkl], axis=1))
    bvl = np.ascontiguousarray(bv.reshape(C.KT, 128).T.astype(bf))
    bo_row = np.ascontiguousarray(bo.reshape(1, C.DM).astype(bf))
    shared = dict(wqt=wqt, wkt=wkt, wvt=wvt, wot=wot, bqkl=bqkl,
                  bvl=bvl, bo_row=bo_row)
    in_maps = []
    B = query.shape[0]
    halves = query.shape[1] // C.SQ
    key_bf = [np.ascontiguousarray(key[b].astype(bf)) for b in range(B)]
    val_bf = [np.ascontiguousarray(value[b].astype(bf)) for b in range(B)]
    for c in range(B * halves):
        b, h = divmod(c, halves)
        m = dict(shared)
        m["xq"] = np.ascontiguousarray(
            query[b, h * C.SQ:(h + 1) * C.SQ, :].astype(bf))
        m["xk"] = key_bf[b]
        m["xv"] = val_bf[b]
        m["mask"] = np.ascontiguousarray(
            mask[b, h * C.SQ:(h + 1) * C.SQ, :].astype(bf))
        in_maps.append(m)
    return in_maps


_CACHED = {}


def get_built():
    if "nc" not in _CACHED:
        _CACHED["nc"] = build(Cfg())
    return _CACHED["nc"]


def kernel(query, key, value, mask, Wq, bq, Wk, bk, Wv, bv, Wo, bo):
    cfg = Cfg()
    nc = get_built()
    in_maps = host_prep(query, key, value, mask, Wq, bq, Wk, bk, Wv, bv, Wo, bo, cfg)
    res = run_bass_kernel_spmd(nc, in_maps, core_ids=list(range(N_CORES)))
    B, S, DM = query.shape
    out = np.empty((B, S, DM), np.float32)
    for c in range(N_CORES):
        b, h = divmod(c, 2)
        out[b, h * cfg.SQ:(h + 1) * cfg.SQ, :] = res.results[c]["out"]
    return out


# revision 68
# speedup vs baseline: 1.4420x; 1.0800x over previous
"""MultiHeadedAttention Trainium2 kernel (8-core SPMD, data-parallel).

Sharding: 8 cores = (batch b in 0..3) x (query half in 0..1). Each core
computes out[b, half*1024:(half+1)*1024, :] independently - no collectives.

v2 pipeline (vs v1: no DRAM staging, per-head-pair proj/attention overlap):
  - host casts x/mask to bf16; device DMA-transposes them straight from the
    input DRAM tensors into SBUF (xbar transpose, 2-byte dtype). All
    transposes on the SP queue only (dual-queue transposes corrupt on HW)
    and phase-separated from DMA copies (Tile serializes copy<->transpose
    pairs ~4us each for the xbar hazard). Every DMA gets its own tile
    (multi-DMA fills of one tile serialize on WAW semaphores).
  - V projection runs first (PE-dense) while k/q/mask transposes stream in
  - per head pair hp: K/Q projection chunks for hp+1 are emitted inside the
    attention j-loop of hp (512-wide psums, emitted after the scores), so PE
    never drains while ACT (exp) is busy
  - attention j-loop is software-pipelined: iteration j emits PV halves of
    j-1 (deps long ready), a proj chunk, then scores/exp/mask for j+1 so
    the in-order PE queue never stalls on exp's psum-slot release
  - scoresT [k,q] psum -> exp on ScalarE -> mask multiply on DVE (bf16 2x
    mode) -> PV accumulate (Z via trailing ones column of v)
  - finalize: Z-row copy on ACT, PE-broadcast Z, reciprocal_approx_fast,
    multiply, DMA hop into xattnT [dm, q]
  - epilogue out = xattnT.T @ WoT + R, R = bv@WoT + bo; the DRAM output
    is bf16 (host casts back to f32 - staging was bf16 anyway), epilogue
    psums alternate the s/pv slot tags for a 4-deep rotation
"""
import numpy as np
import ml_dtypes

import concourse.bass as bass
import concourse.mybir as mybir
import concourse.tile as tile
from concourse import bacc
from concourse.bass_utils import run_bass_kernel_spmd

F32 = mybir.dt.float32
BF16 = mybir.dt.bfloat16
FP8 = mybir.dt.float8e4
AF = mybir.ActivationFunctionType
ALU = mybir.AluOpType

N_CORES = 8
DK = 64


def slices(total, chunk):
    return [(s, min(chunk, total - s)) for s in range(0, total, chunk)]


class Cfg:
    def __init__(self, SQ=1024, SK=2048, DM=1024, H=16, max_stage=5):
        assert DM % 128 == 0 and SK % 128 == 0 and SQ % 128 == 0 and H % 2 == 0
        self.SQ, self.SK, self.DM, self.H = SQ, SK, DM, H
        self.KT = DM // 128          # dm contraction chunks
        self.HP = H // 2             # head pairs
        self.NJ = SK // 128          # Sk tiles
        self.SQS = min(1024, SQ)     # attention Sq slice width (2 psum banks)
        self.max_stage = max_stage
        assert SQ % self.SQS == 0
        assert H * DK == DM


def emit_kernel(tc, cfg, io):
    nc = tc.nc
    C = cfg
    xq, xk, xv, msk = io["xq"], io["xk"], io["xv"], io["mask"]
    w_dram = {"q": io["wqt"], "k": io["wkt"], "v": io["wvt"], "o": io["wot"]}
    bqkl, bvl, bo_row = io["bqkl"], io["bvl"], io["bo_row"]
    out = io["out"]
    PS_F = max(C.SQS, 512)

    pools = {}

    def open_pool(name, bufs=1, space="SBUF"):
        pools[name] = tc.alloc_tile_pool(name=name, bufs=bufs, space=space)
        return pools[name]

    persist = open_pool("persist", 1)
    rot = open_pool("rot", 1)
    ps_s = open_pool("ps_s", 2, space="PSUM")
    ps_pv = open_pool("ps_pv", 2, space="PSUM")
    work = open_pool("work", 1)
    poolA = open_pool("A", 1)   # LIFO: A on top so it can release mid-kernel

    # ---------------- persistent tiles ----------------
    # per-DMA-chunk tiles: a multi-DMA fill of ONE tile serializes on Tile's
    # same-tile WAW semaphores (~5us per DMA round trip), so every DMA gets
    # its own tile
    v_sb = persist.tile([128, C.NJ * C.H * 65], BF16, name="v_sb")
    maskT_t = [persist.tile([128, C.SQ], BF16, name=f"maskT{j}")
               for j in range(C.NJ)]
    xkT_t = [persist.tile([128, C.SK], BF16, name=f"xkT{kt}")
             for kt in range(C.KT)]
    xqT_t = [persist.tile([128, C.SQ], BF16, name=f"xqT{kt}")
             for kt in range(C.KT)]
    R_sb = persist.tile([128, C.DM], BF16, name="R_sb")
    bqkl_sb = persist.tile([128, 2 * C.HP], F32, name="bqkl_sb")
    bvl_sb = persist.tile([128, C.KT], BF16, name="bvl_sb")
    bo_sb = persist.tile([1, C.DM], BF16, name="bo_sb")
    onesb = persist.tile([1, 128], BF16, name="onesb")
    onesf = persist.tile([1, 128], F32, name="onesf")

    xvT_t = {(kt, h): poolA.tile([128, C.SK // 2], BF16, name=f"xvT{kt}_{h}")
             for kt in range(C.KT) for h in range(2)}
    wv_sb = poolA.tile([128, C.KT * C.DM], BF16, name="wv_sb")

    v_view = v_sb.rearrange("p (j h c) -> p j h c", j=C.NJ, c=65)

    # rotating double-buffered tiles, memoized so every use of (kind, hp)
    # shares one handle (a second pool.tile() call would alias a NEW tile
    # into the slot instead of reading what the projection wrote)
    _rot_tiles = {}

    def _rot(kind, hp, shape, nbuf=2):
        key = (kind, hp)
        if key not in _rot_tiles:
            _rot_tiles[key] = rot.tile(shape, BF16, name=f"{kind}{hp % nbuf}",
                                       tag=f"{kind}{hp % nbuf}")
        return _rot_tiles[key]

    def kT_buf(hp):
        return _rot("kT", hp, [128, C.SK])

    def qT_buf(hp):
        return _rot("qT", hp, [128, C.SQ])

    # single-buffered: wkh(hp) is fully consumed by proj(hp) during
    # attention(hp-1), before wkh(hp+1)'s load is issued
    def wkh_buf(hp):
        return _rot("wk", hp, [128, C.KT * 128], nbuf=1)

    def wqh_buf(hp):
        return _rot("wq", hp, [128, C.KT * 128], nbuf=1)

    # ---------------- prologue DMAs ----------------
    nc.gpsimd.dma_start(bqkl_sb[:], bqkl[:])
    nc.gpsimd.dma_start(bvl_sb[:], bvl[:])
    nc.gpsimd.dma_start(bo_sb[:], bo_row[:])
    nc.vector.memset(onesb[:], 1.0)
    nc.vector.memset(onesf[:], 1.0)
    nc.vector.memset(v_view[:, :, :, 64:65], 1.0)

    # DMA phase discipline: copies and transposes must not coexist in the
    # schedule window (Tile serializes every DMACopy<->DmaTranspose pair,
    # ~4us each, to dodge a real HW xbar deadlock). All prologue copies
    # first, then all transposes (SP queue only - dual-queue transposes
    # corrupt data on HW).
    def load_wh(dst, name_w, hp):
        # [1024, 128] column slice -> [128, KT*128] (kt-blocked), one DMA
        nc.sync.dma_start(
            dst.rearrange("p (k c) -> p k c", k=C.KT),
            w_dram[name_w][:, hp * 128:(hp + 1) * 128].rearrange(
                "(k p) c -> p k c", p=128),
        )

    # single big DMAs for weights: fewer copy instructions mean the copy
    # phase fully drains before the transposes (each leftover copy would
    # serialize ~6us pairwise against them)
    nc.scalar.dma_start(
        wv_sb.rearrange("p (k c) -> p k c", k=C.KT),
        w_dram["v"].rearrange("(k p) c -> p k c", p=128),
    )
    load_wh(wkh_buf(0), "k", 0)
    load_wh(wqh_buf(0), "q", 0)

    # xvT in Sk-halves (own tile each): V proj's first k-tiles unblock
    # after 8 transposes instead of all 8 full-Sk ones
    for h, (hs, hw) in enumerate(slices(C.SK, C.SK // 2)):
        for kt in range(C.KT):
            nc.sync.dma_start(
                xvT_t[(kt, h)][:],
                xv[hs:hs + hw, kt * 128:(kt + 1) * 128], transpose=True)
    for kt in range(C.KT):
        nc.sync.dma_start(
            xkT_t[kt][:], xk[:, kt * 128:(kt + 1) * 128], transpose=True)
    for kt in range(C.KT):
        nc.sync.dma_start(
            xqT_t[kt][:], xq[:, kt * 128:(kt + 1) * 128], transpose=True)

    # ---------------- V projection (PE-dense pipeline fill) ----------------
    def v_proj_j(j):
        ps = ps_s.tile([128, C.DM], F32, name="ps_v", tag="s",
                       padded_shape=[128, PS_F])
        h, jo = divmod(j * 128, C.SK // 2)
        for (ds_, dw) in slices(C.DM, 512):
            for kt in range(C.KT):
                nc.tensor.matmul(
                    ps[:, ds_:ds_ + dw],
                    xvT_t[(kt, h)][:, jo:jo + 128],
                    wv_sb[:, kt * C.DM + ds_:kt * C.DM + ds_ + dw],
                    start=(kt == 0), stop=(kt == C.KT - 1),
                )
        nc.vector.tensor_copy(
            v_view[:, j, :, 0:64],
            ps.rearrange("p (h c) -> p h c", c=DK),
        )

    for j in range(C.NJ):
        v_proj_j(j)

    # mask: bf16 transposes from DRAM, one per-j tile each (no WAW chain)
    for j in range(C.NJ):
        nc.sync.dma_start(maskT_t[j][:], msk[:, j * 128:(j + 1) * 128],
                          transpose=True)

    poolA.release()
    del pools["A"]
    poolB = open_pool("B", 1)
    xattnT = poolB.tile([128, C.HP * C.SQ], BF16, name="xattnT")
    wo_sb = poolB.tile([128, C.KT * C.DM], BF16, name="wo_sb")
    nc.gpsimd.dma_start(
        wo_sb.rearrange("p (k c) -> p k c", k=C.KT),
        w_dram["o"].rearrange("(k p) c -> p k c", p=128),
    )

    if C.max_stage <= 2:
        for pl in reversed(list(pools.values())):
            pl.release()
        return

    # ---------------- projections (emitted per head pair) ----------------
    # projection psums are 512 wide: short "s"-slot holds so the scores/exp
    # rotation stalls at most ~1.7us when a proj tile steals a slot
    def _proj_chunk(xT_t, w, dst, bias_col, ns, nw):
        ps = ps_s.tile([128, nw], F32, name="ps_kp", tag="s",
                       padded_shape=[128, PS_F])
        for kt in range(C.KT):
            nc.tensor.matmul(
                ps[:],
                w[:, kt * 128:(kt + 1) * 128],
                xT_t[kt][:, ns: ns + nw],
                start=(kt == 0), stop=(kt == C.KT - 1),
            )
        nc.vector.tensor_scalar_add(out=dst[:, ns:ns + nw], in0=ps[:],
                                    scalar1=bias_col)

    def proj_chunks(hp, kT, wk, qT, wq):
        """Closures emitting one 512-wide projection chunk each (4 K + 2 Q)."""
        out = []
        for (ns, nw) in slices(C.SK, 512):
            out.append(lambda ns=ns, nw=nw: _proj_chunk(
                xkT_t, wk, kT, bqkl_sb[:, C.HP + hp:C.HP + hp + 1], ns, nw))
        for (ns, nw) in slices(C.SQ, 512):
            out.append(lambda ns=ns, nw=nw: _proj_chunk(
                xqT_t, wq, qT, bqkl_sb[:, hp:hp + 1], ns, nw))
        return out

    def k_proj(hp, kT, wk):
        for (ns, nw) in slices(C.SK, 512):
            _proj_chunk(xkT_t, wk, kT, bqkl_sb[:, C.HP + hp:C.HP + hp + 1], ns, nw)

    def q_proj(hp, qT, wq):
        for (ns, nw) in slices(C.SQ, 512):
            _proj_chunk(xqT_t, wq, qT, bqkl_sb[:, hp:hp + 1], ns, nw)

    def emit_R():
        # R = bv@WoT + bo, PE-broadcast to 128 rows
        psR = ps_s.tile([1, C.DM], F32, name="psR", tag="s",
                        padded_shape=[128, PS_F])
        for (ns, nw) in slices(C.DM, 512):
            for kt in range(C.KT):
                nc.tensor.matmul(
                    psR[0:1, ns:ns + nw], bvl_sb[:, kt:kt + 1],
                    wo_sb[:, kt * C.DM + ns:kt * C.DM + ns + nw],
                    start=(kt == 0), stop=(kt == C.KT - 1),
                )
        Rrow = work.tile([1, C.DM], BF16, name="Rrow", tag="zrow", bufs=1,
                         padded_shape=[1, max(C.DM, C.SQS)])
        nc.vector.tensor_tensor(out=Rrow[:], in0=psR[:], in1=bo_sb[:],
                                op=ALU.add)
        psB = ps_s.tile([128, C.DM], F32, name="psB", tag="s",
                        padded_shape=[128, PS_F])
        for (ns, nw) in slices(C.DM, 512):
            nc.tensor.matmul(psB[:, ns:ns + nw], onesb[0:1, :],
                             Rrow[0:1, ns:ns + nw], start=True, stop=True)
        nc.vector.tensor_copy(R_sb[:], psB[:])

    k_proj(0, kT_buf(0), wkh_buf(0))
    q_proj(0, qT_buf(0), wqh_buf(0))

    # ---------------- attention, one head pair at a time ----------------
    carried = {}   # pre-emitted first scores/exp/mask of the next head pair
    for hp in range(C.HP):
        kT = kT_buf(hp)
        qT = qT_buf(hp)
        if hp + 1 < C.HP:
            load_wh(wkh_buf(hp + 1), "k", hp + 1)
            load_wh(wqh_buf(hp + 1), "q", hp + 1)
            nxt_proj = proj_chunks(hp + 1, kT_buf(hp + 1), wkh_buf(hp + 1),
                                   qT_buf(hp + 1), wqh_buf(hp + 1))
        else:
            nxt_proj = []
        for (sq, sw) in slices(C.SQ, C.SQS):
            pv = [
                ps_pv.tile([65, sw], F32, name=f"ps_pv{i}", tag="pv",
                           padded_shape=[65, PS_F])
                for i in range(2)
            ]
            PIPE = 2
            pm_hist = []

            def emit_head(j, i, hp=hp, kT=kT, qT=qT, sq=sq, sw=sw):
                """Scores MMs + exp + mask for one (j, head)."""
                ss = ps_s.tile([128, sw], F32, name=f"ps_sc{i}", tag="s",
                               padded_shape=[128, PS_F])
                for (qs, qw) in slices(sw, 512):
                    nc.tensor.matmul(
                        ss[:, qs:qs + qw],
                        kT[i * 64:(i + 1) * 64, j * 128:(j + 1) * 128],
                        qT[i * 64:(i + 1) * 64, sq + qs: sq + qs + qw],
                        start=True, stop=True,
                    )
                pe = work.tile([128, sw], BF16, name="p_exp", tag="pe",
                               bufs=2, padded_shape=[128, C.SQS])
                nc.scalar.activation(pe[:], ss[:], AF.Exp)
                pm = work.tile([128, sw], BF16, name="p_msk", tag="pm",
                               bufs=5, padded_shape=[128, C.SQS])
                nc.vector.tensor_tensor(
                    out=pm[:], in0=pe[:],
                    in1=maskT_t[j][:, sq: sq + sw],
                    op=ALU.mult,
                )
                return pm

            def emit_pv_half(jj, pmi, i, pv=pv, hp=hp, sw=sw):
                for (qs, qw) in slices(sw, 512):
                    nc.tensor.matmul(
                        pv[i][:, qs:qs + qw], v_view[:, jj, 2 * hp + i, :],
                        pmi[:, qs:qs + qw],
                        start=(jj == 0), stop=(jj == C.NJ - 1),
                    )

            def finalize_head(i, pv=pv, hp=hp, sq=sq, sw=sw):
                # copy PV rows out of PSUM immediately (DVE) and the Z row on
                # ACT, so the pv accumulator slot frees for the next head
                # pair ~3us sooner; normalize from the SBUF copies
                pvn = pv[i][0:64, :]
                zrow = work.tile([1, sw], BF16, name="zrow", tag="zrow",
                                 bufs=1, padded_shape=[1, max(C.DM, C.SQS)])
                nc.scalar.copy(zrow[0:1, :], pv[i][64:65, :])
                zb = ps_s.tile([64, sw], F32, name="zb", tag="s",
                               padded_shape=[128, PS_F])
                for (qs, qw) in slices(sw, 512):
                    nc.tensor.matmul(zb[:, qs:qs + qw], onesb[0:1, 0:64],
                                     zrow[0:1, qs:qs + qw],
                                     start=True, stop=True)
                zr = work.tile([64, sw], F32, name="zr", tag="zr", bufs=1,
                               padded_shape=[64, C.SQS])
                nc.vector.reciprocal_approx_fast(out=zr[:], in_=zb[:])
                tmp = work.tile([64, sw], BF16, name="xat_t", tag="xat_t",
                                bufs=1, padded_shape=[64, C.SQS])
                nc.vector.tensor_tensor(out=tmp[:], in0=pvn,
                                        in1=zr[:], op=ALU.mult)
                nc.sync.dma_start(
                    xattnT[64 * i:64 * (i + 1), hp * C.SQ + sq: hp * C.SQ + sq + sw],
                    tmp[:],
                )

            # software-pipelined emission, PE queue order per iteration:
            # [PV halves (deps long ready), proj chunk, scores j+1] so the
            # head-of-queue never stalls on exp's psum-slot release
            if hp in carried:
                pm_hist.append((0, carried.pop(hp)))
            else:
                pm_hist.append((0, [emit_head(0, 0), emit_head(0, 1)]))
            for j in range(C.NJ):
                if len(pm_hist) >= PIPE:
                    jj, pp = pm_hist.pop(0)
                    emit_pv_half(jj, pp[0], 0)
                    emit_pv_half(jj, pp[1], 1)
                if j == 3 and hp == 1:
                    emit_R()
                if j + 1 < C.NJ:
                    pms_n = [emit_head(j + 1, 0), emit_head(j + 1, 1)]
                    pm_hist.append((j + 1, pms_n))
                # overlap next head pair's projections with this attention;
                # emitted AFTER the scores so ACT's next exp input is never
                # queued behind a 1.7us proj burst
                if j >= 4 and j % 2 == 0 and nxt_proj:
                    nxt_proj.pop(0)()
            # drain: finalize each head right after its last PV half, and
            # pre-emit the next head pair's first scores so ACT's exp
            # stream doesn't idle across the boundary
            for idx, (jj, pp) in enumerate(pm_hist):
                last = idx == len(pm_hist) - 1
                emit_pv_half(jj, pp[0], 0)
                if last:
                    finalize_head(0)
                    if hp + 1 < C.HP:
                        carried[hp + 1] = [emit_head(
                            0, 0, kT=kT_buf(hp + 1), qT=qT_buf(hp + 1))]
                emit_pv_half(jj, pp[1], 1)
                if last:
                    finalize_head(1)
                    if hp + 1 < C.HP:
                        carried[hp + 1].append(emit_head(
                            0, 1, kT=kT_buf(hp + 1), qT=qT_buf(hp + 1)))

    if C.max_stage <= 3:
        for pl in reversed(list(pools.values())):
            pl.release()
        return

    # ---------------- epilogue: output projection ----------------
    for m in range(C.SQ // 128):
        # alternate "s"/"pv" slots: 4-deep psum rotation, and the first
        # tiles use the free "s" banks instead of waiting on hp7's pv
        ps = (ps_s if m % 2 == 0 else ps_pv).tile(
            [128, C.DM], F32, name="ps_o", tag="s" if m % 2 == 0 else "pv",
            padded_shape=[128, PS_F])
        for (qs, qw) in slices(C.DM, 512):
            for hp in range(C.HP):
                nc.tensor.matmul(
                    ps[:, qs:qs + qw],
                    xattnT[:, hp * C.SQ + m * 128: hp * C.SQ + (m + 1) * 128],
                    wo_sb[:, hp * C.DM + qs:hp * C.DM + qs + qw],
                    start=(hp == 0), stop=(hp == C.HP - 1),
                )
        ot = work.tile([128, C.DM], BF16, name="out_sb", tag="out_sb", bufs=2,
                       padded_shape=[128, PS_F])
        nc.vector.tensor_tensor(out=ot[:], in0=ps[:], in1=R_sb[:], op=ALU.add)
        nc.sync.dma_start(out[m * 128:(m + 1) * 128, :], ot[:])

    for pl in reversed(list(pools.values())):
        pl.release()


def build(cfg, reps=1):
    nc = bacc.Bacc("TRN2", target_bir_lowering=False, debug=False)
    C = cfg
    io = {
        "xq": nc.dram_tensor("xq", [C.SQ, C.DM], BF16, kind="ExternalInput").ap(),
        "xk": nc.dram_tensor("xk", [C.SK, C.DM], BF16, kind="ExternalInput").ap(),
        "xv": nc.dram_tensor("xv", [C.SK, C.DM], BF16, kind="ExternalInput").ap(),
        "mask": nc.dram_tensor("mask", [C.SQ, C.SK], BF16, kind="ExternalInput").ap(),
        "wqt": nc.dram_tensor("wqt", [C.DM, C.DM], BF16, kind="ExternalInput").ap(),
        "wkt": nc.dram_tensor("wkt", [C.DM, C.DM], BF16, kind="ExternalInput").ap(),
        "wvt": nc.dram_tensor("wvt", [C.DM, C.DM], BF16, kind="ExternalInput").ap(),
        "wot": nc.dram_tensor("wot", [C.DM, C.DM], BF16, kind="ExternalInput").ap(),
        "bqkl": nc.dram_tensor("bqkl", [128, 2 * C.HP], F32, kind="ExternalInput").ap(),
        "bvl": nc.dram_tensor("bvl", [128, C.KT], BF16, kind="ExternalInput").ap(),
        "bo_row": nc.dram_tensor("bo_row", [1, C.DM], BF16, kind="ExternalInput").ap(),
        "out": nc.dram_tensor("out", [C.SQ, C.DM], BF16, kind="ExternalOutput").ap(),
    }
    with tile.TileContext(nc) as tc:
        for _ in range(reps):
            emit_kernel(tc, cfg, io)
    nc.compile()
    return nc


def host_prep(query, key, value, mask, Wq, bq, Wk, bk, Wv, bv, Wo, bo, cfg):
    """Host-side layout prep (weight transpose/cast, bf16 casts, slicing)."""
    C = cfg
    bf = ml_dtypes.bfloat16
    wqt = np.ascontiguousarray((Wq.T * 0.125).astype(bf))   # 1/sqrt(dk) folded
    wkt = np.ascontiguousarray(Wk.T.astype(bf))
    wvt = np.ascontiguousarray(Wv.T.astype(bf))
    wot = np.ascontiguousarray(Wo.T.astype(bf))
    bql = (bq * 0.125).reshape(C.HP, 128).T.astype(np.float32)
    bkl = bk.reshape(C.HP, 128).T.astype(np.float32)
    bqkl = np.ascontiguousarray(np.concatenate([bql, bkl], axis=1))
    bvl = np.ascontiguousarray(bv.reshape(C.KT, 128).T.astype(bf))
    bo_row = np.ascontiguousarray(bo.reshape(1, C.DM).astype(bf))
    shared = dict(wqt=wqt, wkt=wkt, wvt=wvt, wot=wot, bqkl=bqkl,
                  bvl=bvl, bo_row=bo_row)
    in_maps = []
    B = query.shape[0]
    halves = query.shape[1] // C.SQ
    key_bf = [np.ascontiguousarray(key[b].astype(bf)) for b in range(B)]
    val_bf = [np.ascontiguousarray(value[b].astype(bf)) for b in range(B)]
    for c in range(B * halves):
        b, h = divmod(c, halves)
        m = dict(shared)
        m["xq"] = np.ascontiguousarray(
            query[b, h * C.SQ:(h + 1) * C.SQ, :].astype(bf))
        m["xk"] = key_bf[b]
        m["xv"] = val_bf[b]
        m["mask"] = np.ascontiguousarray(
            mask[b, h * C.SQ:(h + 1) * C.SQ, :].astype(bf))
        in_maps.append(m)
    return in_maps


_CACHED = {}


def get_built():
    if "nc" not in _CACHED:
        _CACHED["nc"] = build(Cfg())
    return _CACHED["nc"]


def kernel(query, key, value, mask, Wq, bq, Wk, bk, Wv, bv, Wo, bo):
    cfg = Cfg()
    nc = get_built()
    in_maps = host_prep(query, key, value, mask, Wq, bq, Wk, bk, Wv, bv, Wo, bo, cfg)
    res = run_bass_kernel_spmd(nc, in_maps, core_ids=list(range(N_CORES)))
    B, S, DM = query.shape
    out = np.empty((B, S, DM), np.float32)
    for c in range(N_CORES):
        b, h = divmod(c, 2)
        out[b, h * cfg.SQ:(h + 1) * cfg.SQ, :] = \
            res.results[c]["out"].astype(np.float32)
    return out


# revision 73
# speedup vs baseline: 1.6342x; 1.1333x over previous
"""MultiHeadedAttention Trainium2 kernel (8-core SPMD, data-parallel).

Sharding: 8 cores = (batch b in 0..3) x (query half in 0..1). Each core
computes out[b, half*1024:(half+1)*1024, :] independently - no collectives.

v2 pipeline (vs v1: no DRAM staging, per-head-pair proj/attention overlap):
  - host casts x/mask to bf16; device DMA-transposes them straight from the
    input DRAM tensors into SBUF (xbar transpose, 2-byte dtype). All
    transposes on the SP queue only (dual-queue transposes corrupt on HW)
    and phase-separated from DMA copies (Tile serializes copy<->transpose
    pairs ~4us each for the xbar hazard). Every DMA gets its own tile
    (multi-DMA fills of one tile serialize on WAW semaphores).
  - V projection runs first (PE-dense) while k/q/mask transposes stream in
  - per head pair hp: K/Q projection chunks for hp+1 are emitted inside the
    attention j-loop of hp (512-wide psums, emitted after the scores), so PE
    never drains while ACT (exp) is busy
  - attention j-loop is software-pipelined: iteration j emits PV halves of
    j-1 (deps long ready), a proj chunk, then scores/exp/mask for j+1 so
    the in-order PE queue never stalls on exp's psum-slot release
  - scoresT [k,q] psum -> exp on ScalarE -> mask multiply on DVE (bf16 2x
    mode) -> PV accumulate (Z via trailing ones column of v)
  - finalize: Z-row copy on ACT, PE-broadcast Z, reciprocal_approx_fast,
    multiply, DMA hop into xattnT [dm, q]
  - epilogue out = xattnT.T @ WoT + R, R = bv@WoT + bo; the DRAM output
    is bf16 (host casts back to f32 - staging was bf16 anyway), epilogue
    psums alternate the s/pv slot tags for a 4-deep rotation
"""
import numpy as np
import ml_dtypes

import concourse.bass as bass
import concourse.mybir as mybir
import concourse.tile as tile
from concourse import bacc
from concourse.bass_utils import run_bass_kernel_spmd

F32 = mybir.dt.float32
BF16 = mybir.dt.bfloat16
FP8 = mybir.dt.float8e4
AF = mybir.ActivationFunctionType
ALU = mybir.AluOpType

N_CORES = 8
DK = 64


def slices(total, chunk):
    return [(s, min(chunk, total - s)) for s in range(0, total, chunk)]


class Cfg:
    def __init__(self, SQ=1024, SK=2048, DM=1024, H=16, max_stage=5):
        assert DM % 128 == 0 and SK % 128 == 0 and SQ % 128 == 0 and H % 2 == 0
        self.SQ, self.SK, self.DM, self.H = SQ, SK, DM, H
        self.KT = DM // 128          # dm contraction chunks
        self.HP = H // 2             # head pairs
        self.NJ = SK // 128          # Sk tiles
        self.SQS = min(1024, SQ)     # attention Sq slice width (2 psum banks)
        self.max_stage = max_stage
        assert SQ % self.SQS == 0
        assert H * DK == DM


def emit_kernel(tc, cfg, io):
    nc = tc.nc
    C = cfg
    xq, xk, xv, msk = io["xq"], io["xk"], io["xv"], io["mask"]
    w_dram = {"q": io["wqt"], "k": io["wkt"], "v": io["wvt"], "o": io["wot"]}
    bqkl, bvl, bo_row = io["bqkl"], io["bvl"], io["bo_row"]
    out = io["out"]
    PS_F = max(C.SQS, 512)

    pools = {}

    def open_pool(name, bufs=1, space="SBUF"):
        pools[name] = tc.alloc_tile_pool(name=name, bufs=bufs, space=space)
        return pools[name]

    persist = open_pool("persist", 1)
    rot = open_pool("rot", 1)
    ps_s = open_pool("ps_s", 2, space="PSUM")
    ps_pv = open_pool("ps_pv", 2, space="PSUM")
    work = open_pool("work", 1)
    poolA = open_pool("A", 1)   # LIFO: A on top so it can release mid-kernel

    # ---------------- persistent tiles ----------------
    # per-DMA-chunk tiles: a multi-DMA fill of ONE tile serializes on Tile's
    # same-tile WAW semaphores (~5us per DMA round trip), so every DMA gets
    # its own tile
    v_sb = persist.tile([128, C.NJ * C.H * 65], BF16, name="v_sb")
    maskT_t = [persist.tile([128, C.SQ], BF16, name=f"maskT{j}")
               for j in range(C.NJ)]
    xkT_t = [persist.tile([128, C.SK], BF16, name=f"xkT{kt}")
             for kt in range(C.KT)]
    xqT_t = [persist.tile([128, C.SQ], BF16, name=f"xqT{kt}")
             for kt in range(C.KT)]
    R_sb = persist.tile([128, C.DM], BF16, name="R_sb")
    bqkl_sb = persist.tile([128, 2 * C.HP], F32, name="bqkl_sb")
    bvl_sb = persist.tile([128, C.KT], BF16, name="bvl_sb")
    bo_sb = persist.tile([1, C.DM], BF16, name="bo_sb")
    onesb = persist.tile([1, 128], BF16, name="onesb")
    onesf = persist.tile([1, 128], F32, name="onesf")

    xvT_t = {(kt, h): poolA.tile([128, C.SK // 2], BF16, name=f"xvT{kt}_{h}")
             for kt in range(C.KT) for h in range(2)}
    wv_sb = poolA.tile([128, C.KT * C.DM], BF16, name="wv_sb")

    v_view = v_sb.rearrange("p (j h c) -> p j h c", j=C.NJ, c=65)

    # rotating double-buffered tiles, memoized so every use of (kind, hp)
    # shares one handle (a second pool.tile() call would alias a NEW tile
    # into the slot instead of reading what the projection wrote)
    _rot_tiles = {}

    def _rot(kind, hp, shape, nbuf=2):
        key = (kind, hp)
        if key not in _rot_tiles:
            _rot_tiles[key] = rot.tile(shape, BF16, name=f"{kind}{hp % nbuf}",
                                       tag=f"{kind}{hp % nbuf}")
        return _rot_tiles[key]

    def kT_buf(hp):
        return _rot("kT", hp, [128, C.SK])

    def qT_buf(hp):
        return _rot("qT", hp, [128, C.SQ])

    # single-buffered: wkh(hp) is fully consumed by proj(hp) during
    # attention(hp-1), before wkh(hp+1)'s load is issued
    def wkh_buf(hp):
        return _rot("wk", hp, [128, C.KT * 128], nbuf=1)

    def wqh_buf(hp):
        return _rot("wq", hp, [128, C.KT * 128], nbuf=1)

    # ---------------- prologue DMAs ----------------
    nc.gpsimd.dma_start(bqkl_sb[:], bqkl[:])
    nc.gpsimd.dma_start(bvl_sb[:], bvl[:])
    nc.gpsimd.dma_start(bo_sb[:], bo_row[:])
    nc.vector.memset(onesb[:], 1.0)
    nc.vector.memset(onesf[:], 1.0)
    nc.vector.memset(v_view[:, :, :, 64:65], 1.0)

    # DMA phase discipline: copies and transposes must not coexist in the
    # schedule window (Tile serializes every DMACopy<->DmaTranspose pair,
    # ~4us each, to dodge a real HW xbar deadlock). All prologue copies
    # first, then all transposes (SP queue only - dual-queue transposes
    # corrupt data on HW).
    def load_wh(dst, name_w, hp, eng=None):
        # [1024, 128] column slice -> [128, KT*128] (kt-blocked), one DMA
        (eng or nc.sync).dma_start(
            dst.rearrange("p (k c) -> p k c", k=C.KT),
            w_dram[name_w][:, hp * 128:(hp + 1) * 128].rearrange(
                "(k p) c -> p k c", p=128),
        )

    # single big DMAs for weights: fewer copy instructions mean the copy
    # phase fully drains before the transposes (each leftover copy would
    # serialize ~6us pairwise against them)
    nc.scalar.dma_start(
        wv_sb.rearrange("p (k c) -> p k c", k=C.KT),
        w_dram["v"].rearrange("(k p) c -> p k c", p=128),
    )
    load_wh(wkh_buf(0), "k", 0)
    load_wh(wqh_buf(0), "q", 0)

    # xvT in Sk-halves (own tile each): V proj's first k-tiles unblock
    # after 8 transposes instead of all 8 full-Sk ones
    for h, (hs, hw) in enumerate(slices(C.SK, C.SK // 2)):
        for kt in range(C.KT):
            nc.sync.dma_start(
                xvT_t[(kt, h)][:],
                xv[hs:hs + hw, kt * 128:(kt + 1) * 128], transpose=True)
    for kt in range(C.KT):
        nc.sync.dma_start(
            xkT_t[kt][:], xk[:, kt * 128:(kt + 1) * 128], transpose=True)
    for kt in range(C.KT):
        nc.sync.dma_start(
            xqT_t[kt][:], xq[:, kt * 128:(kt + 1) * 128], transpose=True)

    # ---------------- V projection (PE-dense pipeline fill) ----------------
    def v_proj_j(j):
        ps = ps_s.tile([128, C.DM], F32, name="ps_v", tag="s",
                       padded_shape=[128, PS_F])
        h, jo = divmod(j * 128, C.SK // 2)
        for (ds_, dw) in slices(C.DM, 512):
            for kt in range(C.KT):
                nc.tensor.matmul(
                    ps[:, ds_:ds_ + dw],
                    xvT_t[(kt, h)][:, jo:jo + 128],
                    wv_sb[:, kt * C.DM + ds_:kt * C.DM + ds_ + dw],
                    start=(kt == 0), stop=(kt == C.KT - 1),
                )
        nc.vector.tensor_copy(
            v_view[:, j, :, 0:64],
            ps.rearrange("p (h c) -> p h c", c=DK),
        )

    for j in range(C.NJ):
        v_proj_j(j)

    # mask: bf16 transposes from DRAM, one per-j tile each (no WAW chain)
    for j in range(C.NJ):
        nc.sync.dma_start(maskT_t[j][:], msk[:, j * 128:(j + 1) * 128],
                          transpose=True)

    poolA.release()
    del pools["A"]
    poolB = open_pool("B", 1)
    xattnT = poolB.tile([128, C.HP * C.SQ], BF16, name="xattnT")
    wo_sb = poolB.tile([128, C.KT * C.DM], BF16, name="wo_sb")
    nc.gpsimd.dma_start(
        wo_sb.rearrange("p (k c) -> p k c", k=C.KT),
        w_dram["o"].rearrange("(k p) c -> p k c", p=128),
    )

    if C.max_stage <= 2:
        for pl in reversed(list(pools.values())):
            pl.release()
        return

    # ---------------- projections (emitted per head pair) ----------------
    # projection psums are 512 wide: short "s"-slot holds so the scores/exp
    # rotation stalls at most ~1.7us when a proj tile steals a slot
    def _proj_chunk(xT_t, w, dst, bias_col, ns, nw):
        ps = ps_s.tile([128, nw], F32, name="ps_kp", tag="s",
                       padded_shape=[128, PS_F])
        for kt in range(C.KT):
            nc.tensor.matmul(
                ps[:],
                w[:, kt * 128:(kt + 1) * 128],
                xT_t[kt][:, ns: ns + nw],
                start=(kt == 0), stop=(kt == C.KT - 1),
            )
        nc.vector.tensor_scalar_add(out=dst[:, ns:ns + nw], in0=ps[:],
                                    scalar1=bias_col)

    def proj_chunks(hp, kT, wk, qT, wq):
        """Closures emitting one 512-wide projection chunk each (4 K + 2 Q)."""
        out = []
        for (ns, nw) in slices(C.SK, 512):
            out.append(lambda ns=ns, nw=nw: _proj_chunk(
                xkT_t, wk, kT, bqkl_sb[:, C.HP + hp:C.HP + hp + 1], ns, nw))
        for (ns, nw) in slices(C.SQ, 512):
            out.append(lambda ns=ns, nw=nw: _proj_chunk(
                xqT_t, wq, qT, bqkl_sb[:, hp:hp + 1], ns, nw))
        return out

    def k_proj(hp, kT, wk):
        for (ns, nw) in slices(C.SK, 512):
            _proj_chunk(xkT_t, wk, kT, bqkl_sb[:, C.HP + hp:C.HP + hp + 1], ns, nw)

    def q_proj(hp, qT, wq):
        for (ns, nw) in slices(C.SQ, 512):
            _proj_chunk(xqT_t, wq, qT, bqkl_sb[:, hp:hp + 1], ns, nw)

    def emit_R():
        # R = bv@WoT + bo, PE-broadcast to 128 rows
        psR = ps_s.tile([1, C.DM], F32, name="psR", tag="s",
                        padded_shape=[128, PS_F])
        for (ns, nw) in slices(C.DM, 512):
            for kt in range(C.KT):
                nc.tensor.matmul(
                    psR[0:1, ns:ns + nw], bvl_sb[:, kt:kt + 1],
                    wo_sb[:, kt * C.DM + ns:kt * C.DM + ns + nw],
                    start=(kt == 0), stop=(kt == C.KT - 1),
                )
        Rrow = work.tile([1, C.DM], BF16, name="Rrow", tag="zrow", bufs=1,
                         padded_shape=[1, max(C.DM, C.SQS)])
        nc.vector.tensor_tensor(out=Rrow[:], in0=psR[:], in1=bo_sb[:],
                                op=ALU.add)
        psB = ps_s.tile([128, C.DM], F32, name="psB", tag="s",
                        padded_shape=[128, PS_F])
        for (ns, nw) in slices(C.DM, 512):
            nc.tensor.matmul(psB[:, ns:ns + nw], onesb[0:1, :],
                             Rrow[0:1, ns:ns + nw], start=True, stop=True)
        nc.vector.tensor_copy(R_sb[:], psB[:])

    k_proj(0, kT_buf(0), wkh_buf(0))
    q_proj(0, qT_buf(0), wqh_buf(0))

    # ---------------- attention, one head pair at a time ----------------
    carried = {}   # pre-emitted first scores/exp/mask of the next head pair
    for hp in range(C.HP):
        kT = kT_buf(hp)
        qT = qT_buf(hp)
        if hp + 1 < C.HP:
            load_wh(wkh_buf(hp + 1), "k", hp + 1)
            load_wh(wqh_buf(hp + 1), "q", hp + 1)
            nxt_proj = proj_chunks(hp + 1, kT_buf(hp + 1), wkh_buf(hp + 1),
                                   qT_buf(hp + 1), wqh_buf(hp + 1))
        else:
            nxt_proj = []
        for (sq, sw) in slices(C.SQ, C.SQS):
            pv = [
                ps_pv.tile([65, sw], F32, name=f"ps_pv{i}", tag="pv",
                           padded_shape=[65, PS_F])
                for i in range(2)
            ]
            PIPE = 2
            pm_hist = []

            def emit_head(j, i, hp=hp, kT=kT, qT=qT, sq=sq, sw=sw):
                """Scores MMs + exp + mask for one (j, head)."""
                ss = ps_s.tile([128, sw], F32, name=f"ps_sc{i}", tag="s",
                               padded_shape=[128, PS_F])
                for (qs, qw) in slices(sw, 512):
                    nc.tensor.matmul(
                        ss[:, qs:qs + qw],
                        kT[i * 64:(i + 1) * 64, j * 128:(j + 1) * 128],
                        qT[i * 64:(i + 1) * 64, sq + qs: sq + qs + qw],
                        start=True, stop=True,
                    )
                pe = work.tile([128, sw], BF16, name="p_exp", tag="pe",
                               bufs=2, padded_shape=[128, C.SQS])
                nc.scalar.activation(pe[:], ss[:], AF.Exp)
                pm = work.tile([128, sw], BF16, name="p_msk", tag="pm",
                               bufs=5, padded_shape=[128, C.SQS])
                nc.vector.tensor_tensor(
                    out=pm[:], in0=pe[:],
                    in1=maskT_t[j][:, sq: sq + sw],
                    op=ALU.mult,
                )
                return pm

            def emit_pv_half(jj, pmi, i, pv=pv, hp=hp, sw=sw):
                for (qs, qw) in slices(sw, 512):
                    nc.tensor.matmul(
                        pv[i][:, qs:qs + qw], v_view[:, jj, 2 * hp + i, :],
                        pmi[:, qs:qs + qw],
                        start=(jj == 0), stop=(jj == C.NJ - 1),
                    )

            def finalize_head(i, pv=pv, hp=hp, sq=sq, sw=sw):
                # copy PV rows out of PSUM immediately (DVE) and the Z row on
                # ACT, so the pv accumulator slot frees for the next head
                # pair ~3us sooner; normalize from the SBUF copies
                pvn = pv[i][0:64, :]
                zrow = work.tile([1, sw], BF16, name="zrow", tag="zrow",
                                 bufs=1, padded_shape=[1, max(C.DM, C.SQS)])
                nc.scalar.copy(zrow[0:1, :], pv[i][64:65, :])
                zb = ps_s.tile([64, sw], F32, name="zb", tag="s",
                               padded_shape=[128, PS_F])
                for (qs, qw) in slices(sw, 512):
                    nc.tensor.matmul(zb[:, qs:qs + qw], onesb[0:1, 0:64],
                                     zrow[0:1, qs:qs + qw],
                                     start=True, stop=True)
                zr = work.tile([64, sw], F32, name="zr", tag="zr", bufs=1,
                               padded_shape=[64, C.SQS])
                nc.vector.reciprocal_approx_fast(out=zr[:], in_=zb[:])
                tmp = work.tile([64, sw], BF16, name="xat_t", tag="xat_t",
                                bufs=1, padded_shape=[64, C.SQS])
                nc.vector.tensor_tensor(out=tmp[:], in0=pvn,
                                        in1=zr[:], op=ALU.mult)
                nc.sync.dma_start(
                    xattnT[64 * i:64 * (i + 1), hp * C.SQ + sq: hp * C.SQ + sq + sw],
                    tmp[:],
                )

            # software-pipelined emission, PE queue order per iteration:
            # [PV halves (deps long ready), proj chunk, scores j+1] so the
            # head-of-queue never stalls on exp's psum-slot release
            if hp in carried:
                pm_hist.append((0, carried.pop(hp)))
            else:
                pm_hist.append((0, [emit_head(0, 0), emit_head(0, 1)]))
            for j in range(C.NJ):
                if len(pm_hist) >= PIPE:
                    jj, pp = pm_hist.pop(0)
                    emit_pv_half(jj, pp[0], 0)
                    emit_pv_half(jj, pp[1], 1)
                if j == 3 and hp == 1:
                    emit_R()
                if j + 1 < C.NJ:
                    pms_n = [emit_head(j + 1, 0), emit_head(j + 1, 1)]
                    pm_hist.append((j + 1, pms_n))
                # overlap next head pair's projections with this attention;
                # emitted AFTER the scores so ACT's next exp input is never
                # queued behind a 1.7us proj burst
                if j >= 4 and j % 2 == 0 and nxt_proj:
                    nxt_proj.pop(0)()
            # drain: finalize each head right after its last PV half, and
            # pre-emit the next head pair's first scores so ACT's exp
            # stream doesn't idle across the boundary
            for idx, (jj, pp) in enumerate(pm_hist):
                last = idx == len(pm_hist) - 1
                emit_pv_half(jj, pp[0], 0)
                if last:
                    finalize_head(0)
                    if hp + 1 < C.HP:
                        carried[hp + 1] = [emit_head(
                            0, 0, kT=kT_buf(hp + 1), qT=qT_buf(hp + 1))]
                emit_pv_half(jj, pp[1], 1)
                if last:
                    finalize_head(1)
                    if hp + 1 < C.HP:
                        carried[hp + 1].append(emit_head(
                            0, 1, kT=kT_buf(hp + 1), qT=qT_buf(hp + 1)))

    if C.max_stage <= 3:
        for pl in reversed(list(pools.values())):
            pl.release()
        return

    # ---------------- epilogue: output projection ----------------
    for m in range(C.SQ // 128):
        # alternate "s"/"pv" slots: 4-deep psum rotation, and the first
        # tiles use the free "s" banks instead of waiting on hp7's pv
        ps = (ps_s if m % 2 == 0 else ps_pv).tile(
            [128, C.DM], F32, name="ps_o", tag="s" if m % 2 == 0 else "pv",
            padded_shape=[128, PS_F])
        for (qs, qw) in slices(C.DM, 512):
            for hp in range(C.HP):
                nc.tensor.matmul(
                    ps[:, qs:qs + qw],
                    xattnT[:, hp * C.SQ + m * 128: hp * C.SQ + (m + 1) * 128],
                    wo_sb[:, hp * C.DM + qs:hp * C.DM + qs + qw],
                    start=(hp == 0), stop=(hp == C.HP - 1),
                )
        ot = work.tile([128, C.DM], BF16, name="out_sb", tag="out_sb", bufs=2,
                       padded_shape=[128, PS_F])
        nc.vector.tensor_tensor(out=ot[:], in0=ps[:], in1=R_sb[:], op=ALU.add)
        nc.sync.dma_start(out[m * 128:(m + 1) * 128, :], ot[:])

    for pl in reversed(list(pools.values())):
        pl.release()


def build(cfg, reps=1):
    nc = bacc.Bacc("TRN2", target_bir_lowering=False, debug=False)
    C = cfg
    io = {
        "xq": nc.dram_tensor("xq", [C.SQ, C.DM], BF16, kind="ExternalInput").ap(),
        "xk": nc.dram_tensor("xk", [C.SK, C.DM], BF16, kind="ExternalInput").ap(),
        "xv": nc.dram_tensor("xv", [C.SK, C.DM], BF16, kind="ExternalInput").ap(),
        "mask": nc.dram_tensor("mask", [C.SQ, C.SK], BF16, kind="ExternalInput").ap(),
        "wqt": nc.dram_tensor("wqt", [C.DM, C.DM], BF16, kind="ExternalInput").ap(),
        "wkt": nc.dram_tensor("wkt", [C.DM, C.DM], BF16, kind="ExternalInput").ap(),
        "wvt": nc.dram_tensor("wvt", [C.DM, C.DM], BF16, kind="ExternalInput").ap(),
        "wot": nc.dram_tensor("wot", [C.DM, C.DM], BF16, kind="ExternalInput").ap(),
        "bqkl": nc.dram_tensor("bqkl", [128, 2 * C.HP], F32, kind="ExternalInput").ap(),
        "bvl": nc.dram_tensor("bvl", [128, C.KT], BF16, kind="ExternalInput").ap(),
        "bo_row": nc.dram_tensor("bo_row", [1, C.DM], BF16, kind="ExternalInput").ap(),
        "out": nc.dram_tensor("out", [C.SQ, C.DM], BF16, kind="ExternalOutput").ap(),
    }
    with tile.TileContext(nc) as tc:
        for _ in range(reps):
            emit_kernel(tc, cfg, io)
    nc.compile()
    return nc


def host_prep(query, key, value, mask, Wq, bq, Wk, bk, Wv, bv, Wo, bo, cfg):
    """Host-side layout prep (weight transpose/cast, bf16 casts, slicing)."""
    C = cfg
    bf = ml_dtypes.bfloat16
    wqt = np.ascontiguousarray((Wq.T * 0.125).astype(bf))   # 1/sqrt(dk) folded
    wkt = np.ascontiguousarray(Wk.T.astype(bf))
    wvt = np.ascontiguousarray(Wv.T.astype(bf))
    wot = np.ascontiguousarray(Wo.T.astype(bf))
    bql = (bq * 0.125).reshape(C.HP, 128).T.astype(np.float32)
    bkl = bk.reshape(C.HP, 128).T.astype(np.float32)
    bqkl = np.ascontiguousarray(np.concatenate([bql, bkl], axis=1))
    bvl = np.ascontiguousarray(bv.reshape(C.KT, 128).T.astype(bf))
    bo_row = np.ascontiguousarray(bo.reshape(1, C.DM).astype(bf))
    shared = dict(wqt=wqt, wkt=wkt, wvt=wvt, wot=wot, bqkl=bqkl,
                  bvl=bvl, bo_row=bo_row)
    in_maps = []
    B = query.shape[0]
    halves = query.shape[1] // C.SQ
    key_bf = [np.ascontiguousarray(key[b].astype(bf)) for b in range(B)]
    val_bf = [np.ascontiguousarray(value[b].astype(bf)) for b in range(B)]
    for c in range(B * halves):
        b, h = divmod(c, halves)
        m = dict(shared)
        m["xq"] = np.ascontiguousarray(
            query[b, h * C.SQ:(h + 1) * C.SQ, :].astype(bf))
        m["xk"] = key_bf[b]
        m["xv"] = val_bf[b]
        m["mask"] = np.ascontiguousarray(
            mask[b, h * C.SQ:(h + 1) * C.SQ, :].astype(bf))
        in_maps.append(m)
    return in_maps


_CACHED = {}


def get_built():
    if "nc" not in _CACHED:
        _CACHED["nc"] = build(Cfg())
    return _CACHED["nc"]


def kernel(query, key, value, mask, Wq, bq, Wk, bk, Wv, bv, Wo, bo):
    cfg = Cfg()
    nc = get_built()
    in_maps = host_prep(query, key, value, mask, Wq, bq, Wk, bk, Wv, bv, Wo, bo, cfg)
    res = run_bass_kernel_spmd(nc, in_maps, core_ids=list(range(N_CORES)))
    B, S, DM = query.shape
    out = np.empty((B, S, DM), np.float32)
    for c in range(N_CORES):
        b, h = divmod(c, 2)
        out[b, h * cfg.SQ:(h + 1) * cfg.SQ, :] = \
            res.results[c]["out"].astype(np.float32)
    return out
